# revision 15
# baseline (speedup 1.0000x reference)
"""2-layer ConvLSTM2D encoder (nn_Encoder_60129542967) on 8 Trainium2 cores.

v2: fp8 DoubleRow matmuls on a flat padded-row stream.

Sharding: data-parallel over batch (B=8 -> 1 batch/core), conv weights
replicated. Output = final (h, c) of each layer: [2, 2, B, 64, 64, 64].

Per core:
- States in flat padded layout [128 ch, 1+66*66+1] fp8e4m3; gates are
  computed over the "stream" = padded rows 1..64 (4224 positions incl.
  the 2 pad cols/row, re-zeroed after each h-write), chunked 8x512+128.
  Every 3x3 tap window is a contiguous run at offset (dy-1)*66+(dx-1),
  so DoubleRow rhs APs are [K, 2, N] as the ISA wants.
- Matmuls: fp8 DoubleRow (2 k-tiles/inst, 0.5 cyc/row). L0 weights are
  error-compensated (e4m3 hi + e5m2 lo, 5 insts/half incl. the x-conv
  hi/lo in the E/Elo planes); L1 is single e4m3 (5 insts/half).
  PSUM accumulates fp32.
- Gate math: hs-gates via ts(min(psum+b',1)) then stt(max(0,.)*V);
  g/tanh(c) on ACT; h = stt(max(0,O2)*TC) written to U as fp8.
  L0: A=[f;i], B=[o;g], V0=[c0;G0]; L1: A=[i;f], B=[g;o], V1=[G1;c1];
  O2=[o0;o1], TC=[tanh c0; tanh c1] so the h-write is one 128p op.
- L0 recurrent convs read DQ planes (D=[h;h>>1], Q=[h;h>>66], E/Elo
  with x hi/lo rows) rebuilt per phase from U by wide DMAs.
- Phases p=1..11 run L1(t=p-1) and L0(t=p) concurrently.
"""

import numpy as np
import ml_dtypes

E4 = ml_dtypes.float8_e4m3
E5 = ml_dtypes.float8_e5m2

B, T, H, W, Cin, F = 8, 10, 64, 64, 1, 64
HP = WP = 66
NPAD = HP * WP           # 4356
NB = 1 + NPAD + 1        # 4358 state-tile cols (guard elements)
SL = 64 * 66             # 4224 stream positions
SB = 67                  # tile offset of stream start
CHUNKS = [(SB + 512 * u, 512) for u in range(8)] + [(SB + 4096, 128)]
R_DQ = 4 * NB
PD, PQ, PE, PEl = 0, NB, 2 * NB, 3 * NB

IDX_I, IDX_F, IDX_G, IDX_O = 0, 1, 2, 3
A0_COLS = np.r_[IDX_F * F:(IDX_F + 1) * F, IDX_I * F:(IDX_I + 1) * F]
B0_COLS = np.r_[IDX_O * F:(IDX_O + 1) * F, IDX_G * F:(IDX_G + 1) * F]
A1_COLS = np.r_[IDX_I * F:(IDX_I + 1) * F, IDX_F * F:(IDX_F + 1) * F]
B1_COLS = np.r_[IDX_G * F:(IDX_G + 1) * F, IDX_O * F:(IDX_O + 1) * F]
SC_A0 = np.full(128, 0.2, np.float32)
SC_B0 = np.r_[np.full(64, 0.2), np.ones(64)].astype(np.float32)
SC_A1 = np.full(128, 0.2, np.float32)
SC_B1 = np.r_[np.ones(64), np.full(64, 0.2)].astype(np.float32)

TAPS = [(dy, dx) for dy in range(3) for dx in range(3)]
L1_PAIRS = [(0, 1), (2, 3), (4, 5), (6, 7), (None, 8)]


def q4(a):
    return np.asarray(a, np.float32).astype(E4).astype(np.float32)


def q5(a):
    return np.asarray(a, np.float32).astype(E5).astype(np.float32)


def win_off(cu, dy, dx):
    return cu + (dy - 1) * 66 + (dx - 1)


def pack_l1(Wx1, Wh1, cols, fold):
    """-> [128, 5*2*128] e4m3 (lhsT slab; k-tile-major per inst)."""
    w_full = np.zeros((9, 128, 128), np.float32)
    for t, (dy, dx) in enumerate(TAPS):
        w_full[t, 0:64] = Wx1[dy, dx][:, cols] * fold
        w_full[t, 64:128] = Wh1[dy, dx][:, cols] * fold
    slab = np.zeros((128, 5, 2, 128), np.float32)
    for i, (a, b) in enumerate(L1_PAIRS):
        if a is not None:
            slab[:, i, 0] = w_full[a]
        if b is not None:
            slab[:, i, 1] = w_full[b]
    return slab.reshape(128, 5 * 256).astype(E4)


def pack_l0(Wx0, Wh0, cols, fold):
    """-> (e4 slab [128, 2*256] e4m3, e5 slab [128, 3*256] e5m2)."""
    wh = {t: Wh0[dy, dx][:, cols] * fold for t, (dy, dx) in enumerate(TAPS)}
    wx = np.stack([Wx0[dy, dx][0][cols] * fold for (dy, dx) in TAPS])

    def ktD(dy):
        return np.concatenate([wh[dy * 3 + 0], wh[dy * 3 + 1]])
    ktQ = np.concatenate([wh[2], wh[5]])

    e4 = np.zeros((128, 2, 2, 128), np.float32)
    e4[:, 0, 0] = q4(ktD(0))
    e4[:, 0, 1] = q4(ktD(1))
    t22_hi = q4(wh[8])
    wx_hi = q4(wx)
    e4[0:64, 1, 0] = t22_hi
    e4[64:73, 1, 0] = wx_hi
    e4[64:73, 1, 1] = wx_hi
    e4[73:82, 1, 1] = q4(wx - wx_hi)

    e5 = np.zeros((128, 3, 2, 128), np.float32)
    d2h = q5(ktD(2))
    qh_ = q5(ktQ)
    e5[:, 0, 0] = d2h
    e5[:, 0, 1] = qh_
    e5[:, 1, 0] = q5(ktD(2) - d2h)
    e5[:, 1, 1] = q5(ktQ - qh_)
    e5[:, 2, 0] = q5(ktD(0) - q4(ktD(0)))
    e5[:, 2, 1] = q5(ktD(1) - q4(ktD(1)))
    return (e4.reshape(128, 2 * 256).astype(E4),
            e5.reshape(128, 3 * 256).astype(E5))


def pack_weights(Wx0, Wh0, b0, Wx1, Wh1, b1):
    out = {
        "wA1": pack_l1(Wx1, Wh1, A1_COLS, SC_A1),
        "wB1": pack_l1(Wx1, Wh1, B1_COLS, SC_B1),
    }
    out["wA0e4"], out["wA0e5"] = pack_l0(Wx0, Wh0, A0_COLS, SC_A0)
    out["wB0e4"], out["wB0e5"] = pack_l0(Wx0, Wh0, B0_COLS, SC_B0)

    def hsb(b):
        return (0.2 * b + 0.5).astype(np.float32)

    out["biasA0"] = hsb(b0[A0_COLS])[:, None]
    out["biasB0"] = np.r_[hsb(b0[B0_COLS[:64]]), b0[B0_COLS[64:]].astype(np.float32)][:, None].astype(np.float32)
    out["biasA1"] = hsb(b1[A1_COLS])[:, None]
    out["biasB1"] = np.r_[b1[B1_COLS[:64]].astype(np.float32), hsb(b1[B1_COLS[64:]])][:, None].astype(np.float32)
    return out


def build_x9(xb):
    """xb [T,64,64] f32 -> [T, 27, NB] e4m3 stream-aligned x planes."""
    out = np.zeros((T, 27, NB), E4)
    for t in range(T):
        hi_f = q4(xb[t])
        lo_f = q4(xb[t] - hi_f)
        pad_hi = np.pad(hi_f, 1)
        pad_lo = np.pad(lo_f, 1)
        for j, (dy, dx) in enumerate(TAPS):
            for src, row in ((pad_hi, j), (pad_lo, 9 + j), (pad_hi, 18 + j)):
                plane66 = np.zeros((66, 66), np.float32)
                plane66[1:65, 1:65] = src[dy:dy + 64, dx:dx + 64]
                out[t, row, 68:68 + 4290] = plane66.reshape(-1)[0:4290].astype(E4)
    return out


_EMITTED = {}


def _emit():
    if "nc" in _EMITTED:
        return _EMITTED["nc"]

    import concourse.bass as bass
    import concourse.mybir as mybir
    import concourse.tile as tile
    from concourse import bacc
    from concourse.ap import AP

    f32 = mybir.dt.float32
    fp8 = mybir.dt.float8e4
    fp8e5 = mybir.dt.float8e5
    Alu = mybir.AluOpType
    Act = mybir.ActivationFunctionType
    DR = mybir.MatmulPerfMode.DoubleRow

    nc = bacc.Bacc("TRN2", target_bir_lowering=False, debug=False, num_devices=8)

    dp = {}
    for name, shape, dt in [
        ("wA1", [128, 1280], fp8), ("wB1", [128, 1280], fp8),
        ("wA0e4", [128, 512], fp8), ("wB0e4", [128, 512], fp8),
        ("wA0e5", [128, 768], fp8e5), ("wB0e5", [128, 768], fp8e5),
        ("biasA0", [128, 1], f32), ("biasB0", [128, 1], f32),
        ("biasA1", [128, 1], f32), ("biasB1", [128, 1], f32),
        ("x9", [T, 27, NB], fp8), ("zeros", [128, NB], fp8),
    ]:
        dp[name] = nc.declare_dram_parameter(name, shape, dt, isOutput=False)
    out_d = nc.declare_dram_parameter("out", [4, 64, SL], f32, isOutput=True)

    with tile.TileContext(nc) as tc:
        with (
            tc.tile_pool(name="consts", bufs=1) as consts,
            tc.tile_pool(name="spool", bufs=3) as spool,
            tc.tile_pool(name="tpool", bufs=2) as tpool,
            tc.tile_pool(name="opool", bufs=3) as opool,
            tc.tile_pool(name="xpool", bufs=2) as xpool,
            tc.tile_pool(name="psg", bufs=8, space="PSUM") as psg,
        ):
            w_sb = {}
            for nm, cols, dt in [("wA1", 1280, fp8), ("wB1", 1280, fp8),
                                 ("wA0e4", 512, fp8), ("wB0e4", 512, fp8),
                                 ("wA0e5", 768, fp8e5), ("wB0e5", 768, fp8e5)]:
                tl = consts.tile([128, cols], dt, tag=nm, name=nm)
                nc.sync.dma_start(out=tl, in_=dp[nm][:, :])
                w_sb[nm] = tl
            bias_sb = {}
            for nm in ("biasA0", "biasB0", "biasA1", "biasB1"):
                tl = consts.tile([128, 1], f32, tag=nm, name=nm)
                nc.sync.dma_start(out=tl, in_=dp[nm][:, :])
                bias_sb[nm] = tl

            U = [consts.tile([128, NB], fp8, tag=f"U{i}", name=f"U{i}")
                 for i in range(2)]
            DQ = [consts.tile([128, R_DQ], fp8, tag=f"DQ{i}", name=f"DQ{i}")
                  for i in range(2)]
            V0 = consts.tile([128, SL], f32, tag="V0", name="V0")
            V1 = consts.tile([128, SL], f32, tag="V1", name="V1")
            TCt = consts.tile([128, SL], f32, tag="TC", name="TC")

            zr = dp["zeros"][:, :]

            # --- init zeroing + phase-1 x planes ---
            # DQ1 E/Elo planes first (phase-1 critical path), then x9[0]
            for i, base in [(1, PE), (1, PEl)]:
                nc.sync.dma_start(
                    out=DQ[i][:, base:base + NB], in_=zr[:, :])
            x9p = dp["x9"]
            for rows, part0, base in ((slice(0, 9), 64, PE),
                                      (slice(9, 18), 64, PEl),
                                      (slice(18, 27), 73, PEl)):
                nc.sync.dma_start(
                    out=DQ[1][part0:part0 + 9, base:base + NB],
                    in_=x9p[0][rows, :])
            nc.gpsimd.dma_start(out=U[1][:, 0:NB], in_=zr[:, :])
            nc.gpsimd.dma_start(out=U[0][:, 0:NB], in_=zr[:, :])
            for i, base in [(1, PD), (1, PQ), (0, PD), (0, PQ), (0, PE), (0, PEl)]:
                nc.scalar.dma_start(
                    out=DQ[i][:, base:base + NB], in_=zr[:, :])

            def mk_rhs(tl, row_stride, part0, npart, offset, delta, n):
                return AP(tensor=tl[:, :].tensor,
                          offset=int(part0 * row_stride + offset),
                          ap=[[row_stride, npart], [delta, 2], [1, n]])

            def lhsT(slab, i, part0=0, npart=128):
                return slab[part0:part0 + npart, i * 256:(i + 1) * 256].rearrange(
                    "p (two m) -> p two m", two=2)

            def l1_matmuls(ps, slab, Uprev, cu, n):
                for i, (a, b) in enumerate(L1_PAIRS):
                    dya, dxa = TAPS[a] if a is not None else (0, 0)
                    oa = win_off(cu, dya, dxa)
                    if b is None:
                        delta = 66  # zero k1 weights; in-range dummy window
                    else:
                        dyb, dxb = TAPS[b]
                        delta = win_off(cu, dyb, dxb) - oa
                    nc.tensor.matmul(
                        ps[:, 0:n], lhsT(slab, i),
                        mk_rhs(Uprev, NB, 0, 128, oa, delta, n),
                        start=(i == 0), stop=(i == 4), perf_mode=DR)

            def l0_matmuls(ps, s4, s5, DQc, cu, n, p):
                oD = [PD + win_off(cu, dy, 0) for dy in range(3)]
                oQ = PQ + win_off(cu, 0, 2)
                oE = PE + cu + 67
                oEl = PEl + cu + 67
                if p == 1:
                    nc.tensor.matmul(
                        ps[:, 0:n], lhsT(s4, 1, 64, 18),
                        mk_rhs(DQc, R_DQ, 64, 18, oE, oEl - oE, n),
                        start=True, stop=True, perf_mode=DR)
                    return
                insts = [
                    (s4, 0, 0, 128, oD[0], 66),          # (D0, D1)
                    (s4, 1, 0, 82, oE, oEl - oE),        # (E, Elo)
                    (s5, 0, 0, 128, oD[2], oQ - oD[2]),  # (D2h, Qh)
                    (s5, 1, 0, 128, oD[2], oQ - oD[2]),  # (D2l, Ql)
                    (s5, 2, 0, 128, oD[0], 66),          # (D0l, D1l)
                ]
                for j, (slab, i, p0, np_, off, delta) in enumerate(insts):
                    nc.tensor.matmul(
                        ps[:, 0:n], lhsT(slab, i, p0, np_),
                        mk_rhs(DQc, R_DQ, p0, np_, off, delta, n),
                        start=(j == 0), stop=(j == 4), perf_mode=DR)

            def pad_ap(tl, cu, n):
                # pad-col pairs q with q-1 in {0,65} mod 66, q in [cu-1, cu+n-1)
                q0 = cu - 1
                while (q0 - 1) % 66 != 65:
                    q0 += 1
                k = (cu + n - 1 - q0 + 65) // 66
                return AP(tensor=tl[:, :].tensor, offset=int(q0),
                          ap=[[NB, 128], [66, k], [1, 2]]), q0, k

            def zsrc_ap(q0, k):
                return AP(tensor=zr.tensor, offset=int(q0),
                          ap=[[NB, 128], [66, k], [1, 2]])

            # pairs for compute/X/TC; DQ copies grouped at phase end
            PAIRS = [[0, 1], [2, 3], [4, 5], [6, 7], [8]]
            DQGROUPS = [[0, 1, 2, 3], [4, 5, 6, 7], [8]]

            Hf = consts.tile([64, SL], f32, tag="Hf", name="Hf")
            for p in range(1, T + 2):
                DQc, DQn = DQ[p % 2], DQ[(p + 1) % 2]
                Ucur, Uprev = U[p % 2], U[(p - 1) % 2]
                if p <= T - 1:
                    # x9[p] into next DQ buffer (readable once phase p-1 done)
                    dst_x = AP(tensor=DQn[:, :].tensor,
                               offset=int(64 * R_DQ + PE),
                               ap=[[R_DQ, 9], [NB, 2], [1, NB]])
                    src_x = AP(tensor=x9p[:, :, :].tensor,
                               offset=int(p * 27 * NB),
                               ap=[[NB, 9], [9 * NB, 2], [1, NB]])
                    nc.sync.dma_start(out=dst_x, in_=src_x)
                    nc.sync.dma_start(
                        out=DQn[73:82, PEl:PEl + NB],
                        in_=x9p[p][18:27, :])
                def emit_front(pr):
                    chunk_ids = PAIRS[pr]
                    gch = [(off * 512, CHUNKS[u][0], CHUNKS[u][1])
                           for off, u in enumerate(chunk_ids)]
                    gn = sum(c[2] for c in gch)
                    st = {"gch": gch, "gn": gn, "O2s": []}
                    if p >= 2:
                        st["T1"] = tpool.tile([128, 1024], f32, tag="T1", name="T1t")
                        st["X1"] = xpool.tile([128, 1024], f32, tag="X1", name="X1t")
                    if p <= T:
                        st["T0"] = tpool.tile([128, 1024], f32, tag="T0", name="T0t")
                        st["X0"] = xpool.tile([128, 1024], f32, tag="X0", name="X0t")
                    for ci, (off, cu, n) in enumerate(gch):
                        s = cu - SB
                        sl = slice(s, s + n)
                        if p >= 2:
                            A1 = psg.tile([128, 512], f32, tag="gates")
                            B1 = psg.tile([128, 512], f32, tag="gates")
                            l1_matmuls(A1, w_sb["wA1"], Uprev, cu, n)
                            l1_matmuls(B1, w_sb["wB1"], Uprev, cu, n)
                        if p <= T:
                            A0 = psg.tile([128, 512], f32, tag="gates")
                            B0 = psg.tile([128, 512], f32, tag="gates")
                            l0_matmuls(A0, w_sb["wA0e4"], w_sb["wA0e5"],
                                       DQc, cu, n, p)
                            l0_matmuls(B0, w_sb["wB0e4"], w_sb["wB0e5"],
                                       DQc, cu, n, p)
                        if p == 1:
                            nc.vector.memset(V0[0:64, sl], 0.0)
                            nc.vector.memset(V1[64:128, sl], 0.0)
                        O2 = opool.tile([128, 512], f32, tag="O2")
                        st["O2s"].append(O2)
                        if p >= 2:
                            S1 = spool.tile([128, 512], f32, tag="S1")
                            nc.gpsimd.tensor_scalar(
                                S1[:, 0:n], A1[:, 0:n],
                                bias_sb["biasA1"][:, 0:1],
                                1.0, op0=Alu.add, op1=Alu.min)
                            nc.scalar.activation(
                                V1[0:64, sl], B1[0:64, 0:n], Act.Tanh,
                                bias=bias_sb["biasB1"][0:64, 0:1])
                            nc.gpsimd.tensor_scalar(
                                O2[64:128, 0:n], B1[64:128, 0:n],
                                bias_sb["biasB1"][64:128, 0:1],
                                1.0, op0=Alu.add, op1=Alu.min)
                            nc.vector.scalar_tensor_tensor(
                                st["T1"][:, off:off + n], S1[:, 0:n], 0.0,
                                V1[:, sl], op0=Alu.max, op1=Alu.mult)
                        if p <= T:
                            S0 = spool.tile([128, 512], f32, tag="S0")
                            nc.gpsimd.tensor_scalar(
                                S0[:, 0:n], A0[:, 0:n],
                                bias_sb["biasA0"][:, 0:1],
                                1.0, op0=Alu.add, op1=Alu.min)
                            nc.scalar.activation(
                                V0[64:128, sl], B0[64:128, 0:n], Act.Tanh,
                                bias=bias_sb["biasB0"][64:128, 0:1])
                            nc.gpsimd.tensor_scalar(
                                O2[0:64, 0:n], B0[0:64, 0:n],
                                bias_sb["biasB0"][0:64, 0:1],
                                1.0, op0=Alu.add, op1=Alu.min)
                            nc.vector.scalar_tensor_tensor(
                                st["T0"][:, off:off + n], S0[:, 0:n], 0.0,
                                V0[:, sl], op0=Alu.max, op1=Alu.mult)
                    if p >= 2:
                        nc.sync.dma_start(out=st["X1"][64:128, 0:gn],
                                          in_=st["T1"][0:64, 0:gn])
                    if p <= T:
                        nc.sync.dma_start(out=st["X0"][0:64, 0:gn],
                                          in_=st["T0"][64:128, 0:gn])
                    return st

                def emit_back(pr, st):
                    gch = st["gch"]
                    gn = st["gn"]
                    for ci, (off, cu, n) in enumerate(gch):
                        s = cu - SB
                        sl = slice(s, s + n)
                        if p >= 2:
                            nc.gpsimd.tensor_tensor(
                                V1[64:128, sl], st["T1"][64:128, off:off + n],
                                st["X1"][64:128, off:off + n], op=Alu.add)
                        if p <= T:
                            eng = nc.vector if (pr + ci) % 2 == 0 else nc.gpsimd
                            eng.tensor_tensor(
                                V0[0:64, sl], st["T0"][0:64, off:off + n],
                                st["X0"][0:64, off:off + n], op=Alu.add)
                    s0 = gch[0][1] - SB
                    psl = slice(s0, s0 + gn)
                    if p <= T:
                        nc.scalar.activation(TCt[0:64, psl], V0[0:64, psl],
                                             Act.Tanh)
                    if p >= 2:
                        nc.scalar.activation(TCt[64:128, psl], V1[64:128, psl],
                                             Act.Tanh)
                    for ci, (off, cu, n) in enumerate(gch):
                        s = cu - SB
                        sl = slice(s, s + n)
                        O2 = st["O2s"][ci]
                        if p == 1:
                            nc.vector.scalar_tensor_tensor(
                                Ucur[0:64, cu:cu + n], O2[0:64, 0:n], 0.0,
                                TCt[0:64, sl], op0=Alu.max, op1=Alu.mult)
                        elif p <= T:
                            nc.vector.scalar_tensor_tensor(
                                Ucur[:, cu:cu + n], O2[:, 0:n], 0.0,
                                TCt[:, sl], op0=Alu.max, op1=Alu.mult)
                        if p == T:
                            nc.vector.scalar_tensor_tensor(
                                Hf[:, sl], O2[0:64, 0:n], 0.0, TCt[0:64, sl],
                                op0=Alu.max, op1=Alu.mult)
                        if p == T + 1:
                            nc.gpsimd.scalar_tensor_tensor(
                                Hf[:, sl], O2[64:128, 0:n], 0.0,
                                TCt[64:128, sl], op0=Alu.max, op1=Alu.mult)
                    if p <= T and pr in (1, 3, 4):
                        gids = {1: [0, 1, 2, 3], 3: [4, 5, 6, 7], 4: [8]}[pr]
                        glo = CHUNKS[gids[0]][0]
                        ghi = CHUNKS[gids[-1]][0] + CHUNKS[gids[-1]][1]
                        gw = ghi - glo
                        j0 = (glo - 132 + 65) // 66
                        j1 = (ghi - 1 - 132) // 66
                        if j1 >= j0:
                            padap = AP(tensor=Ucur[:, :].tensor,
                                       offset=int(132 + 66 * j0),
                                       ap=[[NB, 128], [66, j1 - j0 + 1], [1, 2]])
                            nc.gpsimd.memset(padap, 0.0)
                        if p <= T - 1:
                            dst_plain = AP(
                                tensor=DQn[:, :].tensor, offset=int(PD + glo),
                                ap=[[R_DQ, 64], [NB, 3], [1, gw]])
                            src_plain = AP(
                                tensor=Ucur[:, :].tensor, offset=int(glo),
                                ap=[[NB, 64], [0, 3], [1, gw]])
                            nc.sync.dma_start(out=dst_plain, in_=src_plain)
                            dst_sh = AP(
                                tensor=DQn[:, :].tensor,
                                offset=int(64 * R_DQ + PD + glo - 1),
                                ap=[[R_DQ, 64], [NB - 65, 2], [1, gw]])
                            src_sh = AP(
                                tensor=Ucur[:, :].tensor, offset=int(glo),
                                ap=[[NB, 64], [0, 2], [1, gw]])
                            nc.sync.dma_start(out=dst_sh, in_=src_sh)

                # software-pipelined emission: front(pr) || back(pr-1)
                prev = None
                for pr in range(len(PAIRS)):
                    st = emit_front(pr)
                    if prev is not None:
                        emit_back(pr - 1, prev)
                    prev = st
                emit_back(len(PAIRS) - 1, prev)
                if p == T:
                    nc.sync.dma_start(out=out_d[0][:, :], in_=Hf[:, :])
                    nc.sync.dma_start(out=out_d[1][:, :], in_=V0[0:64, :])
                if p == T + 1:
                    nc.sync.dma_start(out=out_d[2][:, :], in_=Hf[:, :])
                    nc.sync.dma_start(out=out_d[3][:, :], in_=V1[64:128, :])

    nc.compile()
    _EMITTED["nc"] = nc
    return nc


def build_in_maps(x, Wx0, Wh0, b0, Wx1, Wh1, b1):
    packed = pack_weights(np.asarray(Wx0), np.asarray(Wh0), np.asarray(b0),
                          np.asarray(Wx1), np.asarray(Wh1), np.asarray(b1))
    x = np.asarray(x)
    in_maps = []
    for b in range(B):
        m = dict(packed)
        m["x9"] = build_x9(x[b, :, :, :, 0])
        m["zeros"] = np.zeros((128, NB), E4)
        in_maps.append(m)
    return in_maps


def kernel(x, Wx0, Wh0, b0, Wx1, Wh1, b1):
    from concourse.bass_utils import run_bass_kernel_spmd

    nc = _emit()
    in_maps = build_in_maps(x, Wx0, Wh0, b0, Wx1, Wh1, b1)
    res = run_bass_kernel_spmd(nc, in_maps, list(range(B)))

    out = np.zeros((2, 2, B, H, W, F), np.float32)
    for b in range(B):
        r = res.results[b]["out"]  # [4, 64, 4224]
        for (l, si), arr in (((0, 0), r[0]), ((0, 1), r[1]),
                             ((1, 0), r[2]), ((1, 1), r[3])):
            out[l, si, b] = arr.reshape(64, 64, 66)[:, :, 1:65].transpose(1, 2, 0)
    return out


# revision 16
# speedup vs baseline: 1.1001x; 1.1001x over previous
"""2-layer ConvLSTM2D encoder (nn_Encoder_60129542967) on 8 Trainium2 cores.

v2: fp8 DoubleRow matmuls on a flat padded-row stream.

Sharding: data-parallel over batch (B=8 -> 1 batch/core), conv weights
replicated. Output = final (h, c) of each layer: [2, 2, B, 64, 64, 64].

Per core:
- States in flat padded layout [128 ch, 1+66*66+1] fp8e4m3; gates are
  computed over the "stream" = padded rows 1..64 (4224 positions incl.
  the 2 pad cols/row, re-zeroed after each h-write), chunked 8x512+128.
  Every 3x3 tap window is a contiguous run at offset (dy-1)*66+(dx-1),
  so DoubleRow rhs APs are [K, 2, N] as the ISA wants.
- Matmuls: fp8 DoubleRow (2 k-tiles/inst, 0.5 cyc/row). L0 weights are
  error-compensated (e4m3 hi + e5m2 lo, 5 insts/half incl. the x-conv
  hi/lo in the E/Elo planes); L1 is single e4m3 (5 insts/half).
  PSUM accumulates fp32.
- Gate math: hs-gates via ts(min(psum+b',1)) then stt(max(0,.)*V);
  g/tanh(c) on ACT; h = stt(max(0,O2)*TC) written to U as fp8.
  L0: A=[f;i], B=[o;g], V0=[c0;G0]; L1: A=[i;f], B=[g;o], V1=[G1;c1];
  O2=[o0;o1], TC=[tanh c0; tanh c1] so the h-write is one 128p op.
- L0 recurrent convs read DQ planes (D=[h;h>>1], Q=[h;h>>66], E/Elo
  with x hi/lo rows) rebuilt per phase from U by wide DMAs.
- Phases p=1..11 run L1(t=p-1) and L0(t=p) concurrently.
"""

import numpy as np
import ml_dtypes

E4 = ml_dtypes.float8_e4m3
E5 = ml_dtypes.float8_e5m2

B, T, H, W, Cin, F = 8, 10, 64, 64, 1, 64
HP = WP = 66
NPAD = HP * WP           # 4356
NB = 1 + NPAD + 1        # 4358 state-tile cols (guard elements)
SL = 64 * 66             # 4224 stream positions
SB = 67                  # tile offset of stream start
CHUNKS = [(SB + 512 * u, 512) for u in range(8)] + [(SB + 4096, 128)]
R_DQ = 4 * NB
PD, PQ, PE, PEl = 0, NB, 2 * NB, 3 * NB

IDX_I, IDX_F, IDX_G, IDX_O = 0, 1, 2, 3
A0_COLS = np.r_[IDX_F * F:(IDX_F + 1) * F, IDX_I * F:(IDX_I + 1) * F]
B0_COLS = np.r_[IDX_O * F:(IDX_O + 1) * F, IDX_G * F:(IDX_G + 1) * F]
A1_COLS = np.r_[IDX_I * F:(IDX_I + 1) * F, IDX_F * F:(IDX_F + 1) * F]
B1_COLS = np.r_[IDX_G * F:(IDX_G + 1) * F, IDX_O * F:(IDX_O + 1) * F]
SC_A0 = np.full(128, 0.2, np.float32)
SC_B0 = np.r_[np.full(64, 0.2), np.ones(64)].astype(np.float32)
SC_A1 = np.full(128, 0.2, np.float32)
SC_B1 = np.r_[np.ones(64), np.full(64, 0.2)].astype(np.float32)

TAPS = [(dy, dx) for dy in range(3) for dx in range(3)]
L1_PAIRS = [(0, 1), (2, 3), (4, 5), (6, 7), (None, 8)]


def q4(a):
    return np.asarray(a, np.float32).astype(E4).astype(np.float32)


def q5(a):
    return np.asarray(a, np.float32).astype(E5).astype(np.float32)


def win_off(cu, dy, dx):
    return cu + (dy - 1) * 66 + (dx - 1)


def pack_l1(Wx1, Wh1, cols, fold):
    """-> [128, 5*2*128] e4m3 (lhsT slab; k-tile-major per inst)."""
    w_full = np.zeros((9, 128, 128), np.float32)
    for t, (dy, dx) in enumerate(TAPS):
        w_full[t, 0:64] = Wx1[dy, dx][:, cols] * fold
        w_full[t, 64:128] = Wh1[dy, dx][:, cols] * fold
    slab = np.zeros((128, 5, 2, 128), np.float32)
    for i, (a, b) in enumerate(L1_PAIRS):
        if a is not None:
            slab[:, i, 0] = w_full[a]
        if b is not None:
            slab[:, i, 1] = w_full[b]
    return slab.reshape(128, 5 * 256).astype(E4)


def pack_l0(Wx0, Wh0, cols, fold):
    """-> (e4 slab [128, 2*256] e4m3, e5 slab [128, 3*256] e5m2)."""
    wh = {t: Wh0[dy, dx][:, cols] * fold for t, (dy, dx) in enumerate(TAPS)}
    wx = np.stack([Wx0[dy, dx][0][cols] * fold for (dy, dx) in TAPS])

    def ktD(dy):
        return np.concatenate([wh[dy * 3 + 0], wh[dy * 3 + 1]])
    ktQ = np.concatenate([wh[2], wh[5]])

    e4 = np.zeros((128, 2, 2, 128), np.float32)
    e4[:, 0, 0] = q4(ktD(0))
    e4[:, 0, 1] = q4(ktD(1))
    t22_hi = q4(wh[8])
    wx_hi = q4(wx)
    e4[0:64, 1, 0] = t22_hi
    e4[64:73, 1, 0] = wx_hi
    e4[64:73, 1, 1] = wx_hi
    e4[73:82, 1, 1] = q4(wx - wx_hi)

    e5 = np.zeros((128, 3, 2, 128), np.float32)
    d2h = q5(ktD(2))
    qh_ = q5(ktQ)
    e5[:, 0, 0] = d2h
    e5[:, 0, 1] = qh_
    e5[:, 1, 0] = q5(ktD(2) - d2h)
    e5[:, 1, 1] = q5(ktQ - qh_)
    e5[:, 2, 0] = q5(ktD(0) - q4(ktD(0)))
    e5[:, 2, 1] = q5(ktD(1) - q4(ktD(1)))
    return (e4.reshape(128, 2 * 256).astype(E4),
            e5.reshape(128, 3 * 256).astype(E5))


def pack_weights(Wx0, Wh0, b0, Wx1, Wh1, b1):
    out = {
        "wA1": pack_l1(Wx1, Wh1, A1_COLS, SC_A1),
        "wB1": pack_l1(Wx1, Wh1, B1_COLS, SC_B1),
    }
    out["wA0e4"], out["wA0e5"] = pack_l0(Wx0, Wh0, A0_COLS, SC_A0)
    out["wB0e4"], out["wB0e5"] = pack_l0(Wx0, Wh0, B0_COLS, SC_B0)

    def hsb(b):
        return (0.2 * b + 0.5).astype(np.float32)

    out["biasA0"] = hsb(b0[A0_COLS])[:, None]
    out["biasB0"] = np.r_[hsb(b0[B0_COLS[:64]]), b0[B0_COLS[64:]].astype(np.float32)][:, None].astype(np.float32)
    out["biasA1"] = hsb(b1[A1_COLS])[:, None]
    out["biasB1"] = np.r_[b1[B1_COLS[:64]].astype(np.float32), hsb(b1[B1_COLS[64:]])][:, None].astype(np.float32)
    return out


def build_x9(xb):
    """xb [T,64,64] f32 -> [T, 27, NB] e4m3 stream-aligned x planes."""
    out = np.zeros((T, 27, NB), E4)
    for t in range(T):
        hi_f = q4(xb[t])
        lo_f = q4(xb[t] - hi_f)
        pad_hi = np.pad(hi_f, 1)
        pad_lo = np.pad(lo_f, 1)
        for j, (dy, dx) in enumerate(TAPS):
            for src, row in ((pad_hi, j), (pad_lo, 9 + j), (pad_hi, 18 + j)):
                plane66 = np.zeros((66, 66), np.float32)
                plane66[1:65, 1:65] = src[dy:dy + 64, dx:dx + 64]
                out[t, row, 68:68 + 4290] = plane66.reshape(-1)[0:4290].astype(E4)
    return out


_EMITTED = {}


def _emit():
    if "nc" in _EMITTED:
        return _EMITTED["nc"]

    import concourse.bass as bass
    import concourse.mybir as mybir
    import concourse.tile as tile
    from concourse import bacc
    from concourse.ap import AP

    f32 = mybir.dt.float32
    fp8 = mybir.dt.float8e4
    fp8e5 = mybir.dt.float8e5
    Alu = mybir.AluOpType
    Act = mybir.ActivationFunctionType
    DR = mybir.MatmulPerfMode.DoubleRow

    nc = bacc.Bacc("TRN2", target_bir_lowering=False, debug=False, num_devices=8)

    dp = {}
    for name, shape, dt in [
        ("wA1", [128, 1280], fp8), ("wB1", [128, 1280], fp8),
        ("wA0e4", [128, 512], fp8), ("wB0e4", [128, 512], fp8),
        ("wA0e5", [128, 768], fp8e5), ("wB0e5", [128, 768], fp8e5),
        ("biasA0", [128, 1], f32), ("biasB0", [128, 1], f32),
        ("biasA1", [128, 1], f32), ("biasB1", [128, 1], f32),
        ("x9", [T, 27, NB], fp8), ("zeros", [128, NB], fp8),
    ]:
        dp[name] = nc.declare_dram_parameter(name, shape, dt, isOutput=False)
    out_d = nc.declare_dram_parameter("out", [4, 64, SL], f32, isOutput=True)

    with tile.TileContext(nc) as tc:
        with (
            tc.tile_pool(name="consts", bufs=1) as consts,
            tc.tile_pool(name="spool", bufs=3) as spool,
            tc.tile_pool(name="tpool", bufs=2) as tpool,
            tc.tile_pool(name="opool", bufs=3) as opool,
            tc.tile_pool(name="xpool", bufs=2) as xpool,
            tc.tile_pool(name="psg", bufs=8, space="PSUM") as psg,
        ):
            w_sb = {}
            for nm, cols, dt in [("wA1", 1280, fp8), ("wB1", 1280, fp8),
                                 ("wA0e4", 512, fp8), ("wB0e4", 512, fp8),
                                 ("wA0e5", 768, fp8e5), ("wB0e5", 768, fp8e5)]:
                tl = consts.tile([128, cols], dt, tag=nm, name=nm)
                nc.sync.dma_start(out=tl, in_=dp[nm][:, :])
                w_sb[nm] = tl
            bias_sb = {}
            for nm in ("biasA0", "biasB0", "biasA1", "biasB1"):
                tl = consts.tile([128, 1], f32, tag=nm, name=nm)
                nc.sync.dma_start(out=tl, in_=dp[nm][:, :])
                bias_sb[nm] = tl

            U = [consts.tile([128, NB], fp8, tag=f"U{i}", name=f"U{i}")
                 for i in range(2)]
            DQ = [consts.tile([128, R_DQ], fp8, tag=f"DQ{i}", name=f"DQ{i}")
                  for i in range(2)]
            V0 = consts.tile([128, SL], f32, tag="V0", name="V0")
            V1 = consts.tile([128, SL], f32, tag="V1", name="V1")
            TCt = consts.tile([128, SL], f32, tag="TC", name="TC")

            zr = dp["zeros"][:, :]

            # --- init zeroing + phase-1 x planes ---
            # DQ1 E/Elo planes first (phase-1 critical path), then x9[0]
            for i, base in [(1, PE), (1, PEl)]:
                nc.sync.dma_start(
                    out=DQ[i][:, base:base + NB], in_=zr[:, :])
            x9p = dp["x9"]
            for rows, part0, base in ((slice(0, 9), 64, PE),
                                      (slice(9, 18), 64, PEl),
                                      (slice(18, 27), 73, PEl)):
                nc.sync.dma_start(
                    out=DQ[1][part0:part0 + 9, base:base + NB],
                    in_=x9p[0][rows, :])
            nc.gpsimd.dma_start(out=U[1][:, 0:NB], in_=zr[:, :])
            nc.gpsimd.dma_start(out=U[0][:, 0:NB], in_=zr[:, :])
            for i, base in [(1, PD), (1, PQ), (0, PD), (0, PQ), (0, PE), (0, PEl)]:
                nc.scalar.dma_start(
                    out=DQ[i][:, base:base + NB], in_=zr[:, :])

            def mk_rhs(tl, row_stride, part0, npart, offset, delta, n):
                return AP(tensor=tl[:, :].tensor,
                          offset=int(part0 * row_stride + offset),
                          ap=[[row_stride, npart], [delta, 2], [1, n]])

            def lhsT(slab, i, part0=0, npart=128):
                return slab[part0:part0 + npart, i * 256:(i + 1) * 256].rearrange(
                    "p (two m) -> p two m", two=2)

            def l1_matmuls(ps, slab, Uprev, cu, n):
                for i, (a, b) in enumerate(L1_PAIRS):
                    dya, dxa = TAPS[a] if a is not None else (0, 0)
                    oa = win_off(cu, dya, dxa)
                    if b is None:
                        delta = 66  # zero k1 weights; in-range dummy window
                    else:
                        dyb, dxb = TAPS[b]
                        delta = win_off(cu, dyb, dxb) - oa
                    nc.tensor.matmul(
                        ps[:, 0:n], lhsT(slab, i),
                        mk_rhs(Uprev, NB, 0, 128, oa, delta, n),
                        start=(i == 0), stop=(i == 4), perf_mode=DR)

            def l0_matmuls(ps, s4, s5, DQc, cu, n, p):
                oD = [PD + win_off(cu, dy, 0) for dy in range(3)]
                oQ = PQ + win_off(cu, 0, 2)
                oE = PE + cu + 67
                oEl = PEl + cu + 67
                if p == 1:
                    nc.tensor.matmul(
                        ps[:, 0:n], lhsT(s4, 1, 64, 18),
                        mk_rhs(DQc, R_DQ, 64, 18, oE, oEl - oE, n),
                        start=True, stop=True, perf_mode=DR)
                    return
                insts = [
                    (s4, 0, 0, 128, oD[0], 66),          # (D0, D1)
                    (s4, 1, 0, 82, oE, oEl - oE),        # (E, Elo)
                    (s5, 0, 0, 128, oD[2], oQ - oD[2]),  # (D2h, Qh)
                    (s5, 1, 0, 128, oD[2], oQ - oD[2]),  # (D2l, Ql)
                    (s5, 2, 0, 128, oD[0], 66),          # (D0l, D1l)
                ]
                for j, (slab, i, p0, np_, off, delta) in enumerate(insts):
                    nc.tensor.matmul(
                        ps[:, 0:n], lhsT(slab, i, p0, np_),
                        mk_rhs(DQc, R_DQ, p0, np_, off, delta, n),
                        start=(j == 0), stop=(j == 4), perf_mode=DR)

            def pad_ap(tl, cu, n):
                # pad-col pairs q with q-1 in {0,65} mod 66, q in [cu-1, cu+n-1)
                q0 = cu - 1
                while (q0 - 1) % 66 != 65:
                    q0 += 1
                k = (cu + n - 1 - q0 + 65) // 66
                return AP(tensor=tl[:, :].tensor, offset=int(q0),
                          ap=[[NB, 128], [66, k], [1, 2]]), q0, k

            def zsrc_ap(q0, k):
                return AP(tensor=zr.tensor, offset=int(q0),
                          ap=[[NB, 128], [66, k], [1, 2]])

            # pairs for compute/X/TC; DQ copies grouped at phase end
            PAIRS = [[0, 1], [2, 3], [4, 5], [6, 7], [8]]
            DQGROUPS = [[0, 1, 2, 3], [4, 5, 6, 7], [8]]

            Hf = consts.tile([64, SL], f32, tag="Hf", name="Hf")
            for p in range(1, T + 2):
                DQc, DQn = DQ[p % 2], DQ[(p + 1) % 2]
                Ucur, Uprev = U[p % 2], U[(p - 1) % 2]
                if p <= T - 1:
                    # x9[p] into next DQ buffer (readable once phase p-1 done)
                    dst_x = AP(tensor=DQn[:, :].tensor,
                               offset=int(64 * R_DQ + PE),
                               ap=[[R_DQ, 9], [NB, 2], [1, NB]])
                    src_x = AP(tensor=x9p[:, :, :].tensor,
                               offset=int(p * 27 * NB),
                               ap=[[NB, 9], [9 * NB, 2], [1, NB]])
                    nc.sync.dma_start(out=dst_x, in_=src_x)
                    nc.sync.dma_start(
                        out=DQn[73:82, PEl:PEl + NB],
                        in_=x9p[p][18:27, :])
                def emit_front(pr):
                    chunk_ids = PAIRS[pr]
                    gch = [(off * 512, CHUNKS[u][0], CHUNKS[u][1])
                           for off, u in enumerate(chunk_ids)]
                    gn = sum(c[2] for c in gch)
                    st = {"gch": gch, "gn": gn, "O2s": []}
                    if p >= 2:
                        st["T1"] = tpool.tile([128, 1024], f32, tag="T1", name="T1t")
                        st["X1"] = xpool.tile([128, 1024], f32, tag="X1", name="X1t")
                    if p <= T:
                        st["T0"] = tpool.tile([128, 1024], f32, tag="T0", name="T0t")
                        st["X0"] = xpool.tile([128, 1024], f32, tag="X0", name="X0t")
                    for ci, (off, cu, n) in enumerate(gch):
                        s = cu - SB
                        sl = slice(s, s + n)
                        if p >= 2:
                            A1 = psg.tile([128, 512], f32, tag="gates")
                            B1 = psg.tile([128, 512], f32, tag="gates")
                            l1_matmuls(A1, w_sb["wA1"], Uprev, cu, n)
                            l1_matmuls(B1, w_sb["wB1"], Uprev, cu, n)
                        if p <= T:
                            A0 = psg.tile([128, 512], f32, tag="gates")
                            B0 = psg.tile([128, 512], f32, tag="gates")
                            l0_matmuls(A0, w_sb["wA0e4"], w_sb["wA0e5"],
                                       DQc, cu, n, p)
                            l0_matmuls(B0, w_sb["wB0e4"], w_sb["wB0e5"],
                                       DQc, cu, n, p)
                        if p == 1:
                            nc.vector.memset(V0[0:64, sl], 0.0)
                            nc.vector.memset(V1[64:128, sl], 0.0)
                        O2 = opool.tile([128, 512], f32, tag="O2")
                        st["O2s"].append(O2)
                        if p >= 2:
                            S1 = spool.tile([128, 512], f32, tag="S1")
                            nc.gpsimd.tensor_scalar(
                                S1[:, 0:n], A1[:, 0:n],
                                bias_sb["biasA1"][:, 0:1],
                                1.0, op0=Alu.add, op1=Alu.min)
                            nc.scalar.activation(
                                V1[0:64, sl], B1[0:64, 0:n], Act.Tanh,
                                bias=bias_sb["biasB1"][0:64, 0:1])
                            nc.gpsimd.tensor_scalar(
                                O2[64:128, 0:n], B1[64:128, 0:n],
                                bias_sb["biasB1"][64:128, 0:1],
                                1.0, op0=Alu.add, op1=Alu.min)
                            nc.vector.scalar_tensor_tensor(
                                st["T1"][:, off:off + n], S1[:, 0:n], 0.0,
                                V1[:, sl], op0=Alu.max, op1=Alu.mult)
                        if p <= T:
                            S0 = spool.tile([128, 512], f32, tag="S0")
                            nc.gpsimd.tensor_scalar(
                                S0[:, 0:n], A0[:, 0:n],
                                bias_sb["biasA0"][:, 0:1],
                                1.0, op0=Alu.add, op1=Alu.min)
                            nc.scalar.activation(
                                V0[64:128, sl], B0[64:128, 0:n], Act.Tanh,
                                bias=bias_sb["biasB0"][64:128, 0:1])
                            nc.gpsimd.tensor_scalar(
                                O2[0:64, 0:n], B0[0:64, 0:n],
                                bias_sb["biasB0"][0:64, 0:1],
                                1.0, op0=Alu.add, op1=Alu.min)
                            nc.vector.scalar_tensor_tensor(
                                st["T0"][:, off:off + n], S0[:, 0:n], 0.0,
                                V0[:, sl], op0=Alu.max, op1=Alu.mult)
                    if p >= 2:
                        nc.sync.dma_start(out=st["X1"][64:128, 0:gn],
                                          in_=st["T1"][0:64, 0:gn])
                    if p <= T:
                        nc.sync.dma_start(out=st["X0"][0:64, 0:gn],
                                          in_=st["T0"][64:128, 0:gn])
                    return st

                def emit_back(pr, st):
                    gch = st["gch"]
                    gn = st["gn"]
                    for ci, (off, cu, n) in enumerate(gch):
                        s = cu - SB
                        sl = slice(s, s + n)
                        if p >= 2:
                            nc.gpsimd.tensor_tensor(
                                V1[64:128, sl], st["T1"][64:128, off:off + n],
                                st["X1"][64:128, off:off + n], op=Alu.add)
                        if p <= T:
                            eng = nc.vector if (pr + ci) % 2 == 0 else nc.gpsimd
                            eng.tensor_tensor(
                                V0[0:64, sl], st["T0"][0:64, off:off + n],
                                st["X0"][0:64, off:off + n], op=Alu.add)
                    s0 = gch[0][1] - SB
                    psl = slice(s0, s0 + gn)
                    if p <= T:
                        nc.scalar.activation(TCt[0:64, psl], V0[0:64, psl],
                                             Act.Tanh)
                    if p >= 2:
                        nc.scalar.activation(TCt[64:128, psl], V1[64:128, psl],
                                             Act.Tanh)
                    for ci, (off, cu, n) in enumerate(gch):
                        s = cu - SB
                        sl = slice(s, s + n)
                        O2 = st["O2s"][ci]
                        if p == 1:
                            nc.vector.scalar_tensor_tensor(
                                Ucur[0:64, cu:cu + n], O2[0:64, 0:n], 0.0,
                                TCt[0:64, sl], op0=Alu.max, op1=Alu.mult)
                        elif p <= T:
                            nc.vector.scalar_tensor_tensor(
                                Ucur[:, cu:cu + n], O2[:, 0:n], 0.0,
                                TCt[:, sl], op0=Alu.max, op1=Alu.mult)
                        if p == T:
                            nc.vector.scalar_tensor_tensor(
                                Hf[:, sl], O2[0:64, 0:n], 0.0, TCt[0:64, sl],
                                op0=Alu.max, op1=Alu.mult)
                        if p == T + 1:
                            nc.gpsimd.scalar_tensor_tensor(
                                Hf[:, sl], O2[64:128, 0:n], 0.0,
                                TCt[64:128, sl], op0=Alu.max, op1=Alu.mult)
                    if p <= T and pr in (1, 3, 4):
                        gids = {1: [0, 1, 2, 3], 3: [4, 5, 6, 7], 4: [8]}[pr]
                        glo = CHUNKS[gids[0]][0]
                        ghi = CHUNKS[gids[-1]][0] + CHUNKS[gids[-1]][1]
                        gw = ghi - glo
                        j0 = (glo - 132 + 65) // 66
                        j1 = (ghi - 1 - 132) // 66
                        if j1 >= j0:
                            padap = AP(tensor=Ucur[:, :].tensor,
                                       offset=int(132 + 66 * j0),
                                       ap=[[NB, 128], [66, j1 - j0 + 1], [1, 2]])
                            nc.gpsimd.memset(padap, 0.0)
                        if p <= T - 1:
                            hsrc = Ucur[0:64, glo:glo + gw]
                            for base, shift in ((PD, 0), (PQ, 0), (PE, 0),
                                                (PD, 1), (PQ, 66)):
                                d0 = (base + glo - shift
                                      + (64 * R_DQ if shift else 0))
                                dst = AP(tensor=DQn[:, :].tensor,
                                         offset=int(d0),
                                         ap=[[R_DQ, 64], [1, gw]])
                                nc.sync.dma_start(out=dst, in_=hsrc)

                # software-pipelined emission: front(pr) || back(pr-1)
                prev = None
                for pr in range(len(PAIRS)):
                    st = emit_front(pr)
                    if prev is not None:
                        emit_back(pr - 1, prev)
                    prev = st
                emit_back(len(PAIRS) - 1, prev)
                if p == T:
                    nc.sync.dma_start(out=out_d[0][:, :], in_=Hf[:, :])
                    nc.sync.dma_start(out=out_d[1][:, :], in_=V0[0:64, :])
                if p == T + 1:
                    nc.sync.dma_start(out=out_d[2][:, :], in_=Hf[:, :])
                    nc.sync.dma_start(out=out_d[3][:, :], in_=V1[64:128, :])

    nc.compile()
    _EMITTED["nc"] = nc
    return nc


def build_in_maps(x, Wx0, Wh0, b0, Wx1, Wh1, b1):
    packed = pack_weights(np.asarray(Wx0), np.asarray(Wh0), np.asarray(b0),
                          np.asarray(Wx1), np.asarray(Wh1), np.asarray(b1))
    x = np.asarray(x)
    in_maps = []
    for b in range(B):
        m = dict(packed)
        m["x9"] = build_x9(x[b, :, :, :, 0])
        m["zeros"] = np.zeros((128, NB), E4)
        in_maps.append(m)
    return in_maps


def kernel(x, Wx0, Wh0, b0, Wx1, Wh1, b1):
    from concourse.bass_utils import run_bass_kernel_spmd

    nc = _emit()
    in_maps = build_in_maps(x, Wx0, Wh0, b0, Wx1, Wh1, b1)
    res = run_bass_kernel_spmd(nc, in_maps, list(range(B)))

    out = np.zeros((2, 2, B, H, W, F), np.float32)
    for b in range(B):
        r = res.results[b]["out"]  # [4, 64, 4224]
        for (l, si), arr in (((0, 0), r[0]), ((0, 1), r[1]),
                             ((1, 0), r[2]), ((1, 1), r[3])):
            out[l, si, b] = arr.reshape(64, 64, 66)[:, :, 1:65].transpose(1, 2, 0)
    return out


# revision 19
# speedup vs baseline: 1.4416x; 1.3104x over previous
"""2-layer ConvLSTM2D encoder (nn_Encoder_60129542967) on 8 Trainium2 cores.

v2: fp8 DoubleRow matmuls on a flat padded-row stream.

Sharding: data-parallel over batch (B=8 -> 1 batch/core), conv weights
replicated. Output = final (h, c) of each layer: [2, 2, B, 64, 64, 64].

Per core:
- States in flat padded layout [128 ch, 1+66*66+1] fp8e4m3; gates are
  computed over the "stream" = padded rows 1..64 (4224 positions incl.
  the 2 pad cols/row, re-zeroed after each h-write), chunked 8x512+128.
  Every 3x3 tap window is a contiguous run at offset (dy-1)*66+(dx-1),
  so DoubleRow rhs APs are [K, 2, N] as the ISA wants.
- Matmuls: fp8 DoubleRow (2 k-tiles/inst, 0.5 cyc/row). L0 weights are
  error-compensated (e4m3 hi + e5m2 lo, 5 insts/half incl. the x-conv
  hi/lo in the E/Elo planes); L1 is single e4m3 (5 insts/half).
  PSUM accumulates fp32.
- Gate math: hs-gates via ts(min(psum+b',1)) then stt(max(0,.)*V);
  g/tanh(c) on ACT; h = stt(max(0,O2)*TC) written to U as fp8.
  L0: A=[f;i], B=[o;g], V0=[c0;G0]; L1: A=[i;f], B=[g;o], V1=[G1;c1];
  O2=[o0;o1], TC=[tanh c0; tanh c1] so the h-write is one 128p op.
- L0 recurrent convs read DQ planes (D=[h;h>>1], Q=[h;h>>66], E/Elo
  with x hi/lo rows) rebuilt per phase from U by wide DMAs.
- Phases p=1..11 run L1(t=p-1) and L0(t=p) concurrently.
"""

import numpy as np
import ml_dtypes

E4 = ml_dtypes.float8_e4m3
E5 = ml_dtypes.float8_e5m2

B, T, H, W, Cin, F = 8, 10, 64, 64, 1, 64
HP = WP = 66
NPAD = HP * WP           # 4356
NB = 1 + NPAD + 1        # 4358 state-tile cols (guard elements)
SL = 64 * 66             # 4224 stream positions
SB = 67                  # tile offset of stream start
CHUNKS = [(SB + 512 * u, 512) for u in range(8)] + [(SB + 4096, 128)]

IDX_I, IDX_F, IDX_G, IDX_O = 0, 1, 2, 3
A0_COLS = np.r_[IDX_F * F:(IDX_F + 1) * F, IDX_I * F:(IDX_I + 1) * F]
B0_COLS = np.r_[IDX_O * F:(IDX_O + 1) * F, IDX_G * F:(IDX_G + 1) * F]
A1_COLS = np.r_[IDX_I * F:(IDX_I + 1) * F, IDX_F * F:(IDX_F + 1) * F]
B1_COLS = np.r_[IDX_G * F:(IDX_G + 1) * F, IDX_O * F:(IDX_O + 1) * F]
SC_A0 = np.full(128, 0.2, np.float32)
SC_B0 = np.r_[np.full(64, 0.2), np.ones(64)].astype(np.float32)
SC_A1 = np.full(128, 0.2, np.float32)
SC_B1 = np.r_[np.ones(64), np.full(64, 0.2)].astype(np.float32)

TAPS = [(dy, dx) for dy in range(3) for dx in range(3)]
L1_PAIRS = [(0, 1), (2, 3), (4, 5), (6, 7), (None, 8)]


def q4(a):
    return np.asarray(a, np.float32).astype(E4).astype(np.float32)


def q5(a):
    return np.asarray(a, np.float32).astype(E5).astype(np.float32)


def win_off(cu, dy, dx):
    return cu + (dy - 1) * 66 + (dx - 1)


def pack_l1(Wx1, Wh1, cols, fold):
    """-> [128, 5*2*128] e4m3 (lhsT slab; k-tile-major per inst)."""
    w_full = np.zeros((9, 128, 128), np.float32)
    for t, (dy, dx) in enumerate(TAPS):
        w_full[t, 0:64] = Wx1[dy, dx][:, cols] * fold
        w_full[t, 64:128] = Wh1[dy, dx][:, cols] * fold
    slab = np.zeros((128, 5, 2, 128), np.float32)
    for i, (a, b) in enumerate(L1_PAIRS):
        if a is not None:
            slab[:, i, 0] = w_full[a]
        if b is not None:
            slab[:, i, 1] = w_full[b]
    return slab.reshape(128, 5 * 256).astype(E4)


def pack_l0(Wx0, Wh0, cols, fold):
    """-> (e4 slab [128, 4*256] e4m3, e5 slab [128, 3*256] e5m2).

    hi insts: (P0,P1), (H0,P2), (H1,H2), (xa,xb).  lo: same P/H structure.
    P(dy): parts 0:64 = tap (dy,0), 64:128 = (dy,1) (D-plane pair).
    H(dy): parts 0:64 = tap (dy,2), 64:128 = 0 (half-used D k-tile).
    x-inst (parts 64:82): k0 = [wx-hi(64:73); wx-lo(73:82)] on xa,
    k1 = [wx-hi(64:73); 0] on xb.
    """
    wh = {t: Wh0[dy, dx][:, cols] * fold for t, (dy, dx) in enumerate(TAPS)}
    wx = np.stack([Wx0[dy, dx][0][cols] * fold for (dy, dx) in TAPS])

    def ktP(dy):
        return np.concatenate([wh[dy * 3 + 0], wh[dy * 3 + 1]])

    def ktH(dy):
        z = np.zeros((128, 128), np.float32)
        z[0:64] = wh[dy * 3 + 2]
        return z

    wx_hi = q4(wx)
    wx_lo = q4(wx - wx_hi)
    e4 = np.zeros((128, 4, 2, 128), np.float32)
    e4[:, 0, 0] = q4(ktP(0))
    e4[:, 0, 1] = q4(ktP(1))
    e4[:, 1, 0] = q4(ktH(0))
    e4[:, 1, 1] = q4(ktP(2))
    e4[:, 2, 0] = q4(ktH(1))
    e4[:, 2, 1] = q4(ktH(2))
    e4[64:73, 3, 0] = wx_hi
    e4[73:82, 3, 0] = wx_lo
    e4[64:73, 3, 1] = wx_hi

    e5 = np.zeros((128, 3, 2, 128), np.float32)
    e5[:, 0, 0] = q5(ktP(0) - q4(ktP(0)))
    e5[:, 0, 1] = q5(ktP(1) - q4(ktP(1)))
    e5[:, 1, 0] = q5(ktH(0) - q4(ktH(0)))
    e5[:, 1, 1] = q5(ktP(2) - q4(ktP(2)))
    e5[:, 2, 0] = q5(ktH(1) - q4(ktH(1)))
    e5[:, 2, 1] = q5(ktH(2) - q4(ktH(2)))
    return (e4.reshape(128, 4 * 256).astype(E4),
            e5.reshape(128, 3 * 256).astype(E5))


def pack_weights(Wx0, Wh0, b0, Wx1, Wh1, b1):
    out = {
        "wA1": pack_l1(Wx1, Wh1, A1_COLS, SC_A1),
        "wB1": pack_l1(Wx1, Wh1, B1_COLS, SC_B1),
    }
    out["wA0e4"], out["wA0e5"] = pack_l0(Wx0, Wh0, A0_COLS, SC_A0)
    out["wB0e4"], out["wB0e5"] = pack_l0(Wx0, Wh0, B0_COLS, SC_B0)

    def hsb(b):
        return (0.2 * b + 0.5).astype(np.float32)

    out["biasA0"] = hsb(b0[A0_COLS])[:, None]
    out["biasB0"] = np.r_[hsb(b0[B0_COLS[:64]]), b0[B0_COLS[64:]].astype(np.float32)][:, None].astype(np.float32)
    out["biasA1"] = hsb(b1[A1_COLS])[:, None]
    out["biasB1"] = np.r_[b1[B1_COLS[:64]].astype(np.float32), hsb(b1[B1_COLS[64:]])][:, None].astype(np.float32)
    return out


def build_x9(xb):
    """xb [T,64,64] f32 -> [T, 18, 2*NB] e4m3 stream-aligned x planes.

    row r: cols 0:NB = xa-plane (xhi_r if r<9 else xhi_{r-9} dup),
           cols NB:2NB = xb-plane (xlo_r if r<9 else 0)."""
    out = np.zeros((T, 18, 2 * NB), E4)
    for t in range(T):
        hi_f = q4(xb[t])
        lo_f = q4(xb[t] - hi_f)
        pad_hi = np.pad(hi_f, 1)
        pad_lo = np.pad(lo_f, 1)
        for j, (dy, dx) in enumerate(TAPS):
            plane66 = np.zeros((66, 66), np.float32)
            plane66[1:65, 1:65] = pad_hi[dy:dy + 64, dx:dx + 64]
            hi_flat = plane66.reshape(-1)[0:4290].astype(E4)
            plane66[1:65, 1:65] = pad_lo[dy:dy + 64, dx:dx + 64]
            lo_flat = plane66.reshape(-1)[0:4290].astype(E4)
            out[t, j, 68:68 + 4290] = hi_flat
            out[t, j, NB + 68:NB + 68 + 4290] = lo_flat
            out[t, 9 + j, 68:68 + 4290] = hi_flat
    return out


_EMITTED = {}


def _emit():
    if "nc" in _EMITTED:
        return _EMITTED["nc"]

    import concourse.bass as bass
    import concourse.mybir as mybir
    import concourse.tile as tile
    from concourse import bacc
    from concourse.ap import AP

    f32 = mybir.dt.float32
    bf16 = mybir.dt.bfloat16
    fp8 = mybir.dt.float8e4
    fp8e5 = mybir.dt.float8e5
    Alu = mybir.AluOpType
    Act = mybir.ActivationFunctionType
    DR = mybir.MatmulPerfMode.DoubleRow

    nc = bacc.Bacc("TRN2", target_bir_lowering=False, debug=False, num_devices=8)

    dp = {}
    for name, shape, dt in [
        ("wA1", [128, 1280], fp8), ("wB1", [128, 1280], fp8),
        ("wA0e4", [128, 1024], fp8), ("wB0e4", [128, 1024], fp8),
        ("wA0e5", [128, 768], fp8e5), ("wB0e5", [128, 768], fp8e5),
        ("biasA0", [128, 1], f32), ("biasB0", [128, 1], f32),
        ("biasA1", [128, 1], f32), ("biasB1", [128, 1], f32),
        ("x9", [T, 18, 2 * NB], fp8), ("zeros", [128, NB], fp8),
    ]:
        dp[name] = nc.declare_dram_parameter(name, shape, dt, isOutput=False)
    out_d = nc.declare_dram_parameter("out", [4, 64, SL], f32, isOutput=True)

    with tile.TileContext(nc) as tc:
        with (
            tc.tile_pool(name="consts", bufs=1) as consts,
            tc.tile_pool(name="spool", bufs=3) as spool,
            tc.tile_pool(name="tpool", bufs=2) as tpool,
            tc.tile_pool(name="opool", bufs=3) as opool,
            tc.tile_pool(name="xpool", bufs=2) as xpool,
            tc.tile_pool(name="psg", bufs=8, space="PSUM") as psg,
        ):
            w_sb = {}
            for nm, cols, dt in [("wA1", 1280, fp8), ("wB1", 1280, fp8),
                                 ("wA0e4", 1024, fp8), ("wB0e4", 1024, fp8),
                                 ("wA0e5", 768, fp8e5), ("wB0e5", 768, fp8e5)]:
                tl = consts.tile([128, cols], dt, tag=nm, name=nm)
                nc.sync.dma_start(out=tl, in_=dp[nm][:, :])
                w_sb[nm] = tl
            bias_sb = {}
            for nm in ("biasA0", "biasB0", "biasA1", "biasB1"):
                tl = consts.tile([128, 1], f32, tag=nm, name=nm)
                nc.sync.dma_start(out=tl, in_=dp[nm][:, :])
                bias_sb[nm] = tl

            U = [consts.tile([128, NB], fp8, tag=f"U{i}", name=f"U{i}")
                 for i in range(2)]
            Dt = [consts.tile([128, NB], fp8, tag=f"Dt{i}", name=f"Dt{i}")
                  for i in range(2)]
            XT = [consts.tile([128, 2 * NB], fp8, tag=f"XT{i}", name=f"XT{i}")
                  for i in range(2)]
            V0 = consts.tile([128, SL], f32, tag="V0", name="V0")
            V1 = consts.tile([128, SL], f32, tag="V1", name="V1")
            TCt = consts.tile([128, SL], f32, tag="TC", name="TC")

            zr = dp["zeros"][:, :]

            # --- init zeroing + phase-1 x planes ---
            x9p = dp["x9"]
            nc.sync.dma_start(out=XT[1][64:82, :], in_=x9p[0][:, :])
            nc.gpsimd.dma_start(out=U[1][:, 0:NB], in_=zr[:, :])
            nc.gpsimd.dma_start(out=U[0][:, 0:NB], in_=zr[:, :])
            nc.scalar.dma_start(out=Dt[1][:, 0:NB], in_=zr[:, :])
            nc.scalar.dma_start(out=Dt[0][:, 0:NB], in_=zr[:, :])
            nc.scalar.dma_start(out=Dt[1][:, 0:NB].bitcast(fp8),
                                in_=zr[:, :]) if False else None

            def mk_rhs(tl, row_stride, part0, npart, offset, delta, n):
                return AP(tensor=tl[:, :].tensor,
                          offset=int(part0 * row_stride + offset),
                          ap=[[row_stride, npart], [delta, 2], [1, n]])

            def lhsT(slab, i, part0=0, npart=128):
                return slab[part0:part0 + npart, i * 256:(i + 1) * 256].rearrange(
                    "p (two m) -> p two m", two=2)

            def l1_matmuls(ps, slab, Uprev, cu, n):
                for i, (a, b) in enumerate(L1_PAIRS):
                    dya, dxa = TAPS[a] if a is not None else (0, 0)
                    oa = win_off(cu, dya, dxa)
                    if b is None:
                        delta = 66  # zero k1 weights; in-range dummy window
                    else:
                        dyb, dxb = TAPS[b]
                        delta = win_off(cu, dyb, dxb) - oa
                    nc.tensor.matmul(
                        ps[:, 0:n], lhsT(slab, i),
                        mk_rhs(Uprev, NB, 0, 128, oa, delta, n),
                        start=(i == 0), stop=(i == 4), perf_mode=DR)

            def l0_matmuls(ps, s4, s5, Dc, Xc, cu, n, p):
                oP = [win_off(cu, dy, 0) for dy in range(3)]
                oH = [win_off(cu, dy, 2) for dy in range(3)]
                oX = cu + 67
                if p == 1:
                    nc.tensor.matmul(
                        ps[:, 0:n], lhsT(s4, 3, 64, 18),
                        mk_rhs(Xc, 2 * NB, 64, 18, oX, NB, n),
                        start=True, stop=True, perf_mode=DR)
                    return
                insts = [
                    (s4, 0, Dc, NB, 128, oP[0], 66),
                    (s4, 1, Dc, NB, 128, oH[0], oP[2] - oH[0]),
                    (s4, 2, Dc, NB, 128, oH[1], 66),
                    (s4, 3, Xc, 2 * NB, 18, oX, NB),
                    (s5, 0, Dc, NB, 128, oP[0], 66),
                    (s5, 1, Dc, NB, 128, oH[0], oP[2] - oH[0]),
                    (s5, 2, Dc, NB, 128, oH[1], 66),
                ]
                for j, (slab, i, tl, rs, np_, off, delta) in enumerate(insts):
                    p0 = 64 if np_ == 18 else 0
                    nc.tensor.matmul(
                        ps[:, 0:n], lhsT(slab, i, p0, np_),
                        mk_rhs(tl, rs, p0, np_, off, delta, n),
                        start=(j == 0), stop=(j == len(insts) - 1),
                        perf_mode=DR)

            def pad_ap(tl, cu, n):
                # pad-col pairs q with q-1 in {0,65} mod 66, q in [cu-1, cu+n-1)
                q0 = cu - 1
                while (q0 - 1) % 66 != 65:
                    q0 += 1
                k = (cu + n - 1 - q0 + 65) // 66
                return AP(tensor=tl[:, :].tensor, offset=int(q0),
                          ap=[[NB, 128], [66, k], [1, 2]]), q0, k

            def zsrc_ap(q0, k):
                return AP(tensor=zr.tensor, offset=int(q0),
                          ap=[[NB, 128], [66, k], [1, 2]])

            # pairs for compute/X/TC; DQ copies grouped at phase end
            PAIRS = [[0, 1], [2, 3], [4, 5], [6, 7], [8]]
            DQGROUPS = [[0, 1, 2, 3], [4, 5, 6, 7], [8]]

            Hf = consts.tile([64, SL], f32, tag="Hf", name="Hf")
            for p in range(1, T + 2):
                Dc, Dn = Dt[p % 2], Dt[(p + 1) % 2]
                Xc, Xn = XT[p % 2], XT[(p + 1) % 2]
                Ucur, Uprev = U[p % 2], U[(p - 1) % 2]
                if p <= T - 1:
                    # x9[p] into next X buffer (readable once phase p-1 done)
                    nc.sync.dma_start(out=Xn[64:82, :], in_=x9p[p][:, :])
                def emit_front(pr):
                    chunk_ids = PAIRS[pr]
                    gch = [(off * 512, CHUNKS[u][0], CHUNKS[u][1])
                           for off, u in enumerate(chunk_ids)]
                    gn = sum(c[2] for c in gch)
                    st = {"gch": gch, "gn": gn, "O2s": []}
                    if p >= 2:
                        st["T1"] = tpool.tile([128, 1024], bf16, tag="T1", name="T1t")
                        st["X1"] = xpool.tile([128, 1024], bf16, tag="X1", name="X1t")
                    if p <= T:
                        st["T0"] = tpool.tile([128, 1024], bf16, tag="T0", name="T0t")
                        st["X0"] = xpool.tile([128, 1024], bf16, tag="X0", name="X0t")
                    for ci, (off, cu, n) in enumerate(gch):
                        s = cu - SB
                        sl = slice(s, s + n)
                        if p >= 2:
                            A1 = psg.tile([128, 512], f32, tag="gates")
                            B1 = psg.tile([128, 512], f32, tag="gates")
                            l1_matmuls(A1, w_sb["wA1"], Uprev, cu, n)
                            l1_matmuls(B1, w_sb["wB1"], Uprev, cu, n)
                        if p <= T:
                            A0 = psg.tile([128, 512], f32, tag="gates")
                            B0 = psg.tile([128, 512], f32, tag="gates")
                            l0_matmuls(A0, w_sb["wA0e4"], w_sb["wA0e5"],
                                       Dc, Xc, cu, n, p)
                            l0_matmuls(B0, w_sb["wB0e4"], w_sb["wB0e5"],
                                       Dc, Xc, cu, n, p)
                        if p == 1:
                            nc.vector.memset(V0[0:64, sl], 0.0)
                            nc.vector.memset(V1[64:128, sl], 0.0)
                        O2 = opool.tile([128, 512], f32, tag="O2")
                        st["O2s"].append(O2)
                        if p >= 2:
                            S1 = spool.tile([128, 512], f32, tag="S1")
                            nc.gpsimd.tensor_scalar(
                                S1[:, 0:n], A1[:, 0:n],
                                bias_sb["biasA1"][:, 0:1],
                                1.0, op0=Alu.add, op1=Alu.min)
                            nc.scalar.activation(
                                V1[0:64, sl], B1[0:64, 0:n], Act.Tanh,
                                bias=bias_sb["biasB1"][0:64, 0:1])
                            nc.gpsimd.tensor_scalar(
                                O2[64:128, 0:n], B1[64:128, 0:n],
                                bias_sb["biasB1"][64:128, 0:1],
                                1.0, op0=Alu.add, op1=Alu.min)
                            nc.vector.scalar_tensor_tensor(
                                st["T1"][:, off:off + n], S1[:, 0:n], 0.0,
                                V1[:, sl], op0=Alu.max, op1=Alu.mult)
                        if p <= T:
                            S0 = spool.tile([128, 512], f32, tag="S0")
                            nc.gpsimd.tensor_scalar(
                                S0[:, 0:n], A0[:, 0:n],
                                bias_sb["biasA0"][:, 0:1],
                                1.0, op0=Alu.add, op1=Alu.min)
                            nc.scalar.activation(
                                V0[64:128, sl], B0[64:128, 0:n], Act.Tanh,
                                bias=bias_sb["biasB0"][64:128, 0:1])
                            nc.gpsimd.tensor_scalar(
                                O2[0:64, 0:n], B0[0:64, 0:n],
                                bias_sb["biasB0"][0:64, 0:1],
                                1.0, op0=Alu.add, op1=Alu.min)
                            nc.vector.scalar_tensor_tensor(
                                st["T0"][:, off:off + n], S0[:, 0:n], 0.0,
                                V0[:, sl], op0=Alu.max, op1=Alu.mult)
                    if p >= 2:
                        nc.sync.dma_start(out=st["X1"][64:128, 0:gn],
                                          in_=st["T1"][0:64, 0:gn])
                    if p <= T:
                        nc.sync.dma_start(out=st["X0"][0:64, 0:gn],
                                          in_=st["T0"][64:128, 0:gn])
                    return st

                def emit_back(pr, st):
                    gch = st["gch"]
                    gn = st["gn"]
                    for ci, (off, cu, n) in enumerate(gch):
                        s = cu - SB
                        sl = slice(s, s + n)
                        if p >= 2:
                            nc.gpsimd.tensor_tensor(
                                V1[64:128, sl], st["T1"][64:128, off:off + n],
                                st["X1"][64:128, off:off + n], op=Alu.add)
                        if p <= T:
                            eng = nc.vector if (pr + ci) % 2 == 0 else nc.gpsimd
                            eng.tensor_tensor(
                                V0[0:64, sl], st["T0"][0:64, off:off + n],
                                st["X0"][0:64, off:off + n], op=Alu.add)
                    s0 = gch[0][1] - SB
                    psl = slice(s0, s0 + gn)
                    if p <= T:
                        nc.scalar.activation(TCt[0:64, psl], V0[0:64, psl],
                                             Act.Tanh)
                    if p >= 2:
                        nc.scalar.activation(TCt[64:128, psl], V1[64:128, psl],
                                             Act.Tanh)
                    for ci, (off, cu, n) in enumerate(gch):
                        s = cu - SB
                        sl = slice(s, s + n)
                        O2 = st["O2s"][ci]
                        if p == 1:
                            nc.vector.scalar_tensor_tensor(
                                Ucur[0:64, cu:cu + n], O2[0:64, 0:n], 0.0,
                                TCt[0:64, sl], op0=Alu.max, op1=Alu.mult)
                        elif p <= T:
                            nc.vector.scalar_tensor_tensor(
                                Ucur[:, cu:cu + n], O2[:, 0:n], 0.0,
                                TCt[:, sl], op0=Alu.max, op1=Alu.mult)
                        if p == T:
                            nc.vector.scalar_tensor_tensor(
                                Hf[:, sl], O2[0:64, 0:n], 0.0, TCt[0:64, sl],
                                op0=Alu.max, op1=Alu.mult)
                        if p == T + 1:
                            nc.gpsimd.scalar_tensor_tensor(
                                Hf[:, sl], O2[64:128, 0:n], 0.0,
                                TCt[64:128, sl], op0=Alu.max, op1=Alu.mult)
                    if p <= T and pr in (1, 3, 4):
                        gids = {1: [0, 1, 2, 3], 3: [4, 5, 6, 7], 4: [8]}[pr]
                        glo = CHUNKS[gids[0]][0]
                        ghi = CHUNKS[gids[-1]][0] + CHUNKS[gids[-1]][1]
                        gw = ghi - glo
                        j0 = (glo - 132 + 65) // 66
                        j1 = (ghi - 1 - 132) // 66
                        if j1 >= j0:
                            padap = AP(tensor=Ucur[:, :].tensor,
                                       offset=int(132 + 66 * j0),
                                       ap=[[NB, 128], [66, j1 - j0 + 1], [1, 2]])
                            nc.gpsimd.memset(padap, 0.0)
                        if p <= T - 1:
                            hsrc = Ucur[0:64, glo:glo + gw]
                            nc.sync.dma_start(
                                out=Dn[0:64, glo:glo + gw], in_=hsrc)
                            dst_sh = AP(tensor=Dn[:, :].tensor,
                                        offset=int(64 * NB + glo - 1),
                                        ap=[[NB, 64], [1, gw]])
                            nc.sync.dma_start(out=dst_sh, in_=hsrc)

                # software-pipelined emission: front(pr) || back(pr-1)
                prev = None
                for pr in range(len(PAIRS)):
                    st = emit_front(pr)
                    if prev is not None:
                        emit_back(pr - 1, prev)
                    prev = st
                emit_back(len(PAIRS) - 1, prev)
                if p == T:
                    nc.sync.dma_start(out=out_d[0][:, :], in_=Hf[:, :])
                    nc.sync.dma_start(out=out_d[1][:, :], in_=V0[0:64, :])
                if p == T + 1:
                    nc.sync.dma_start(out=out_d[2][:, :], in_=Hf[:, :])
                    nc.sync.dma_start(out=out_d[3][:, :], in_=V1[64:128, :])

    nc.compile()
    _EMITTED["nc"] = nc
    return nc


def build_in_maps(x, Wx0, Wh0, b0, Wx1, Wh1, b1):
    packed = pack_weights(np.asarray(Wx0), np.asarray(Wh0), np.asarray(b0),
                          np.asarray(Wx1), np.asarray(Wh1), np.asarray(b1))
    x = np.asarray(x)
    in_maps = []
    for b in range(B):
        m = dict(packed)
        m["x9"] = build_x9(x[b, :, :, :, 0])
        m["zeros"] = np.zeros((128, NB), E4)
        in_maps.append(m)
    return in_maps


def kernel(x, Wx0, Wh0, b0, Wx1, Wh1, b1):
    from concourse.bass_utils import run_bass_kernel_spmd

    nc = _emit()
    in_maps = build_in_maps(x, Wx0, Wh0, b0, Wx1, Wh1, b1)
    res = run_bass_kernel_spmd(nc, in_maps, list(range(B)))

    out = np.zeros((2, 2, B, H, W, F), np.float32)
    for b in range(B):
        r = res.results[b]["out"]  # [4, 64, 4224]
        for (l, si), arr in (((0, 0), r[0]), ((0, 1), r[1]),
                             ((1, 0), r[2]), ((1, 1), r[3])):
            out[l, si, b] = arr.reshape(64, 64, 66)[:, :, 1:65].transpose(1, 2, 0)
    return out


# revision 21
# speedup vs baseline: 1.4971x; 1.0385x over previous
"""2-layer ConvLSTM2D encoder (nn_Encoder_60129542967) on 8 Trainium2 cores.

v2: fp8 DoubleRow matmuls on a flat padded-row stream.

Sharding: data-parallel over batch (B=8 -> 1 batch/core), conv weights
replicated. Output = final (h, c) of each layer: [2, 2, B, 64, 64, 64].

Per core:
- States in flat padded layout [128 ch, 1+66*66+1] fp8e4m3; gates are
  computed over the "stream" = padded rows 1..64 (4224 positions incl.
  the 2 pad cols/row, re-zeroed after each h-write), chunked 8x512+128.
  Every 3x3 tap window is a contiguous run at offset (dy-1)*66+(dx-1),
  so DoubleRow rhs APs are [K, 2, N] as the ISA wants.
- Matmuls: fp8 DoubleRow (2 k-tiles/inst, 0.5 cyc/row). L0 weights are
  error-compensated (e4m3 hi + e5m2 lo, 5 insts/half incl. the x-conv
  hi/lo in the E/Elo planes); L1 is single e4m3 (5 insts/half).
  PSUM accumulates fp32.
- Gate math: hs-gates via ts(min(psum+b',1)) then stt(max(0,.)*V);
  g/tanh(c) on ACT; h = stt(max(0,O2)*TC) written to U as fp8.
  L0: A=[f;i], B=[o;g], V0=[c0;G0]; L1: A=[i;f], B=[g;o], V1=[G1;c1];
  O2=[o0;o1], TC=[tanh c0; tanh c1] so the h-write is one 128p op.
- L0 recurrent convs read DQ planes (D=[h;h>>1], Q=[h;h>>66], E/Elo
  with x hi/lo rows) rebuilt per phase from U by wide DMAs.
- Phases p=1..11 run L1(t=p-1) and L0(t=p) concurrently.
"""

import numpy as np
import ml_dtypes

E4 = ml_dtypes.float8_e4m3
E5 = ml_dtypes.float8_e5m2

B, T, H, W, Cin, F = 8, 10, 64, 64, 1, 64
HP = WP = 66
NPAD = HP * WP           # 4356
NB = 1 + NPAD + 1        # 4358 state-tile cols (guard elements)
SL = 64 * 66             # 4224 stream positions
SB = 67                  # tile offset of stream start
CHUNKS = [(SB + 512 * u, 512) for u in range(8)] + [(SB + 4096, 128)]

IDX_I, IDX_F, IDX_G, IDX_O = 0, 1, 2, 3
A0_COLS = np.r_[IDX_F * F:(IDX_F + 1) * F, IDX_I * F:(IDX_I + 1) * F]
B0_COLS = np.r_[IDX_O * F:(IDX_O + 1) * F, IDX_G * F:(IDX_G + 1) * F]
A1_COLS = np.r_[IDX_I * F:(IDX_I + 1) * F, IDX_F * F:(IDX_F + 1) * F]
B1_COLS = np.r_[IDX_G * F:(IDX_G + 1) * F, IDX_O * F:(IDX_O + 1) * F]
SC_A0 = np.full(128, 0.2, np.float32)
SC_B0 = np.r_[np.full(64, 0.2), np.ones(64)].astype(np.float32)
SC_A1 = np.full(128, 0.2, np.float32)
SC_B1 = np.r_[np.ones(64), np.full(64, 0.2)].astype(np.float32)

TAPS = [(dy, dx) for dy in range(3) for dx in range(3)]
L1_PAIRS = [(0, 1), (2, 3), (4, 5), (6, 7), (None, 8)]


def q4(a):
    return np.asarray(a, np.float32).astype(E4).astype(np.float32)


def q5(a):
    return np.asarray(a, np.float32).astype(E5).astype(np.float32)


def win_off(cu, dy, dx):
    return cu + (dy - 1) * 66 + (dx - 1)


def pack_l1(Wx1, Wh1, cols, fold):
    """-> [128, 5*2*128] e4m3 (lhsT slab; k-tile-major per inst)."""
    w_full = np.zeros((9, 128, 128), np.float32)
    for t, (dy, dx) in enumerate(TAPS):
        w_full[t, 0:64] = Wx1[dy, dx][:, cols] * fold
        w_full[t, 64:128] = Wh1[dy, dx][:, cols] * fold
    slab = np.zeros((128, 5, 2, 128), np.float32)
    for i, (a, b) in enumerate(L1_PAIRS):
        if a is not None:
            slab[:, i, 0] = w_full[a]
        if b is not None:
            slab[:, i, 1] = w_full[b]
    return slab.reshape(128, 5 * 256).astype(E4)


def pack_l0(Wx0, Wh0, cols, fold):
    """-> (e4 slab [128, 4*256] e4m3, e5 slab [128, 3*256] e5m2).

    hi insts: (P0,P1), (H0,P2), (H1,H2), (xa,xb).  lo: same P/H structure.
    P(dy): parts 0:64 = tap (dy,0), 64:128 = (dy,1) (D-plane pair).
    H(dy): parts 0:64 = tap (dy,2), 64:128 = 0 (half-used D k-tile).
    x-inst (parts 64:82): k0 = [wx-hi(64:73); wx-lo(73:82)] on xa,
    k1 = [wx-hi(64:73); 0] on xb.
    """
    wh = {t: Wh0[dy, dx][:, cols] * fold for t, (dy, dx) in enumerate(TAPS)}
    wx = np.stack([Wx0[dy, dx][0][cols] * fold for (dy, dx) in TAPS])

    def ktP(dy):
        return np.concatenate([wh[dy * 3 + 0], wh[dy * 3 + 1]])

    def ktH(dy):
        z = np.zeros((128, 128), np.float32)
        z[0:64] = wh[dy * 3 + 2]
        return z

    wx_hi = q4(wx)
    wx_lo = q4(wx - wx_hi)
    e4 = np.zeros((128, 4, 2, 128), np.float32)
    e4[:, 0, 0] = q4(ktP(0))
    e4[:, 0, 1] = q4(ktP(1))
    e4[:, 1, 0] = q4(ktH(0))
    e4[:, 1, 1] = q4(ktP(2))
    e4[:, 2, 0] = q4(ktH(1))
    e4[:, 2, 1] = q4(ktH(2))
    e4[64:73, 3, 0] = wx_hi
    e4[73:82, 3, 0] = wx_lo
    e4[64:73, 3, 1] = wx_hi

    e5 = np.zeros((128, 3, 2, 128), np.float32)
    e5[:, 0, 0] = q5(ktP(0) - q4(ktP(0)))
    e5[:, 0, 1] = q5(ktP(1) - q4(ktP(1)))
    e5[:, 1, 0] = q5(ktH(0) - q4(ktH(0)))
    e5[:, 1, 1] = q5(ktP(2) - q4(ktP(2)))
    e5[:, 2, 0] = q5(ktH(1) - q4(ktH(1)))
    e5[:, 2, 1] = q5(ktH(2) - q4(ktH(2)))
    return (e4.reshape(128, 4 * 256).astype(E4),
            e5.reshape(128, 3 * 256).astype(E5))


def pack_weights(Wx0, Wh0, b0, Wx1, Wh1, b1):
    out = {
        "wA1": pack_l1(Wx1, Wh1, A1_COLS, SC_A1),
        "wB1": pack_l1(Wx1, Wh1, B1_COLS, SC_B1),
    }
    out["wA0e4"], out["wA0e5"] = pack_l0(Wx0, Wh0, A0_COLS, SC_A0)
    out["wB0e4"], out["wB0e5"] = pack_l0(Wx0, Wh0, B0_COLS, SC_B0)

    def hsb(b):
        return (0.2 * b + 0.5).astype(np.float32)

    out["biasA0"] = hsb(b0[A0_COLS])[:, None]
    out["biasB0"] = np.r_[hsb(b0[B0_COLS[:64]]), b0[B0_COLS[64:]].astype(np.float32)][:, None].astype(np.float32)
    out["biasA1"] = hsb(b1[A1_COLS])[:, None]
    out["biasB1"] = np.r_[b1[B1_COLS[:64]].astype(np.float32), hsb(b1[B1_COLS[64:]])][:, None].astype(np.float32)
    return out


def build_x9(xb):
    """xb [T,64,64] f32 -> [T, 18, 2*NB] e4m3 stream-aligned x planes.

    row r: cols 0:NB = xa-plane (xhi_r if r<9 else xhi_{r-9} dup),
           cols NB:2NB = xb-plane (xlo_r if r<9 else 0)."""
    out = np.zeros((T, 18, 2 * NB), E4)
    for t in range(T):
        hi_f = q4(xb[t])
        lo_f = q4(xb[t] - hi_f)
        pad_hi = np.pad(hi_f, 1)
        pad_lo = np.pad(lo_f, 1)
        for j, (dy, dx) in enumerate(TAPS):
            plane66 = np.zeros((66, 66), np.float32)
            plane66[1:65, 1:65] = pad_hi[dy:dy + 64, dx:dx + 64]
            hi_flat = plane66.reshape(-1)[0:4290].astype(E4)
            plane66[1:65, 1:65] = pad_lo[dy:dy + 64, dx:dx + 64]
            lo_flat = plane66.reshape(-1)[0:4290].astype(E4)
            out[t, j, 68:68 + 4290] = hi_flat
            out[t, j, NB + 68:NB + 68 + 4290] = lo_flat
            out[t, 9 + j, 68:68 + 4290] = hi_flat
    return out


_EMITTED = {}


def _emit():
    if "nc" in _EMITTED:
        return _EMITTED["nc"]

    import concourse.bass as bass
    import concourse.mybir as mybir
    import concourse.tile as tile
    from concourse import bacc
    from concourse.ap import AP

    f32 = mybir.dt.float32
    bf16 = mybir.dt.bfloat16
    fp8 = mybir.dt.float8e4
    fp8e5 = mybir.dt.float8e5
    Alu = mybir.AluOpType
    Act = mybir.ActivationFunctionType
    DR = mybir.MatmulPerfMode.DoubleRow

    nc = bacc.Bacc("TRN2", target_bir_lowering=False, debug=False, num_devices=8)

    dp = {}
    for name, shape, dt in [
        ("wA1", [128, 1280], fp8), ("wB1", [128, 1280], fp8),
        ("wA0e4", [128, 1024], fp8), ("wB0e4", [128, 1024], fp8),
        ("wA0e5", [128, 768], fp8e5), ("wB0e5", [128, 768], fp8e5),
        ("biasA0", [128, 1], f32), ("biasB0", [128, 1], f32),
        ("biasA1", [128, 1], f32), ("biasB1", [128, 1], f32),
        ("x9", [T, 18, 2 * NB], fp8), ("zeros", [128, NB], fp8),
    ]:
        dp[name] = nc.declare_dram_parameter(name, shape, dt, isOutput=False)
    out_d = nc.declare_dram_parameter("out", [4, 64, SL], f32, isOutput=True)

    with tile.TileContext(nc) as tc:
        with (
            tc.tile_pool(name="consts", bufs=1) as consts,
            tc.tile_pool(name="spool", bufs=3) as spool,
            tc.tile_pool(name="tpool", bufs=2) as tpool,
            tc.tile_pool(name="opool", bufs=3) as opool,
            tc.tile_pool(name="xpool", bufs=2) as xpool,
            tc.tile_pool(name="psg", bufs=8, space="PSUM") as psg,
        ):
            w_sb = {}
            for nm, cols, dt in [("wA1", 1280, fp8), ("wB1", 1280, fp8),
                                 ("wA0e4", 1024, fp8), ("wB0e4", 1024, fp8),
                                 ("wA0e5", 768, fp8e5), ("wB0e5", 768, fp8e5)]:
                tl = consts.tile([128, cols], dt, tag=nm, name=nm)
                nc.sync.dma_start(out=tl, in_=dp[nm][:, :])
                w_sb[nm] = tl
            bias_sb = {}
            for nm in ("biasA0", "biasB0", "biasA1", "biasB1"):
                tl = consts.tile([128, 1], f32, tag=nm, name=nm)
                nc.sync.dma_start(out=tl, in_=dp[nm][:, :])
                bias_sb[nm] = tl

            U = [consts.tile([128, NB], fp8, tag=f"U{i}", name=f"U{i}")
                 for i in range(2)]
            Dt = [consts.tile([128, NB], fp8, tag=f"Dt{i}", name=f"Dt{i}")
                  for i in range(2)]
            XT = [consts.tile([128, 2 * NB], fp8, tag=f"XT{i}", name=f"XT{i}")
                  for i in range(2)]
            V0 = consts.tile([128, SL], f32, tag="V0", name="V0")
            V1 = consts.tile([128, SL], f32, tag="V1", name="V1")
            TCt = consts.tile([128, SL], f32, tag="TC", name="TC")

            zr = dp["zeros"][:, :]

            # --- init zeroing + phase-1 x planes ---
            x9p = dp["x9"]
            nc.sync.dma_start(out=XT[1][64:82, :], in_=x9p[0][:, :])
            nc.sync.dma_start(out=U[1][:, 0:NB], in_=zr[:, :])
            nc.sync.dma_start(out=Dt[0][:, 0:NB], in_=zr[:, :])
            nc.sync.dma_start(out=U[0][:, 0:NB], in_=zr[:, :])
            nc.sync.dma_start(out=Dt[1][:, 0:NB], in_=zr[:, :])


            def mk_rhs(tl, row_stride, part0, npart, offset, delta, n):
                return AP(tensor=tl[:, :].tensor,
                          offset=int(part0 * row_stride + offset),
                          ap=[[row_stride, npart], [delta, 2], [1, n]])

            def lhsT(slab, i, part0=0, npart=128):
                return slab[part0:part0 + npart, i * 256:(i + 1) * 256].rearrange(
                    "p (two m) -> p two m", two=2)

            def l1_matmuls(ps, slab, Uprev, cu, n):
                for i, (a, b) in enumerate(L1_PAIRS):
                    dya, dxa = TAPS[a] if a is not None else (0, 0)
                    oa = win_off(cu, dya, dxa)
                    if b is None:
                        delta = 66  # zero k1 weights; in-range dummy window
                    else:
                        dyb, dxb = TAPS[b]
                        delta = win_off(cu, dyb, dxb) - oa
                    nc.tensor.matmul(
                        ps[:, 0:n], lhsT(slab, i),
                        mk_rhs(Uprev, NB, 0, 128, oa, delta, n),
                        start=(i == 0), stop=(i == 4), perf_mode=DR)

            def l0_matmuls(ps, s4, s5, Dc, Xc, cu, n, p):
                oP = [win_off(cu, dy, 0) for dy in range(3)]
                oH = [win_off(cu, dy, 2) for dy in range(3)]
                oX = cu + 67
                if p == 1:
                    nc.tensor.matmul(
                        ps[:, 0:n], lhsT(s4, 3, 64, 18),
                        mk_rhs(Xc, 2 * NB, 64, 18, oX, NB, n),
                        start=True, stop=True, perf_mode=DR)
                    return
                insts = [
                    (s4, 0, Dc, NB, 128, oP[0], 66),
                    (s4, 1, Dc, NB, 128, oH[0], oP[2] - oH[0]),
                    (s4, 2, Dc, NB, 128, oH[1], 66),
                    (s4, 3, Xc, 2 * NB, 18, oX, NB),
                    (s5, 0, Dc, NB, 128, oP[0], 66),
                    (s5, 1, Dc, NB, 128, oH[0], oP[2] - oH[0]),
                    (s5, 2, Dc, NB, 128, oH[1], 66),
                ]
                for j, (slab, i, tl, rs, np_, off, delta) in enumerate(insts):
                    p0 = 64 if np_ == 18 else 0
                    nc.tensor.matmul(
                        ps[:, 0:n], lhsT(slab, i, p0, np_),
                        mk_rhs(tl, rs, p0, np_, off, delta, n),
                        start=(j == 0), stop=(j == len(insts) - 1),
                        perf_mode=DR)

            def pad_ap(tl, cu, n):
                # pad-col pairs q with q-1 in {0,65} mod 66, q in [cu-1, cu+n-1)
                q0 = cu - 1
                while (q0 - 1) % 66 != 65:
                    q0 += 1
                k = (cu + n - 1 - q0 + 65) // 66
                return AP(tensor=tl[:, :].tensor, offset=int(q0),
                          ap=[[NB, 128], [66, k], [1, 2]]), q0, k

            def zsrc_ap(q0, k):
                return AP(tensor=zr.tensor, offset=int(q0),
                          ap=[[NB, 128], [66, k], [1, 2]])

            # pairs for compute/X/TC; DQ copies grouped at phase end
            PAIRS = [[0, 1], [2, 3], [4, 5], [6, 7], [8]]
            DQGROUPS = [[0, 1, 2, 3], [4, 5, 6, 7], [8]]

            Hf = consts.tile([64, SL], f32, tag="Hf", name="Hf")
            for p in range(1, T + 2):
                Dc, Dn = Dt[p % 2], Dt[(p + 1) % 2]
                Xc, Xn = XT[p % 2], XT[(p + 1) % 2]
                Ucur, Uprev = U[p % 2], U[(p - 1) % 2]
                if p <= T - 1:
                    # x9[p] into next X buffer (readable once phase p-1 done)
                    nc.sync.dma_start(out=Xn[64:82, :], in_=x9p[p][:, :])
                def emit_front(pr):
                    chunk_ids = PAIRS[pr]
                    gch = [(off * 512, CHUNKS[u][0], CHUNKS[u][1])
                           for off, u in enumerate(chunk_ids)]
                    gn = sum(c[2] for c in gch)
                    st = {"gch": gch, "gn": gn, "O2s": []}
                    if p >= 2:
                        st["T1"] = tpool.tile([128, 1024], bf16, tag="T1", name="T1t")
                        st["X1"] = xpool.tile([128, 1024], bf16, tag="X1", name="X1t")
                    if p <= T:
                        st["T0"] = tpool.tile([128, 1024], bf16, tag="T0", name="T0t")
                        st["X0"] = xpool.tile([128, 1024], bf16, tag="X0", name="X0t")
                    for ci, (off, cu, n) in enumerate(gch):
                        s = cu - SB
                        sl = slice(s, s + n)
                        if p >= 2:
                            A1 = psg.tile([128, 512], f32, tag="gates")
                            B1 = psg.tile([128, 512], f32, tag="gates")
                            l1_matmuls(A1, w_sb["wA1"], Uprev, cu, n)
                            l1_matmuls(B1, w_sb["wB1"], Uprev, cu, n)
                        if p <= T:
                            A0 = psg.tile([128, 512], f32, tag="gates")
                            B0 = psg.tile([128, 512], f32, tag="gates")
                            l0_matmuls(A0, w_sb["wA0e4"], w_sb["wA0e5"],
                                       Dc, Xc, cu, n, p)
                            l0_matmuls(B0, w_sb["wB0e4"], w_sb["wB0e5"],
                                       Dc, Xc, cu, n, p)
                        if p == 1:
                            nc.vector.memset(V0[0:64, sl], 0.0)
                            nc.vector.memset(V1[64:128, sl], 0.0)
                        O2 = opool.tile([128, 512], f32, tag="O2")
                        st["O2s"].append(O2)
                        if p >= 2:
                            S1 = spool.tile([128, 512], f32, tag="S1")
                            nc.gpsimd.tensor_scalar(
                                S1[:, 0:n], A1[:, 0:n],
                                bias_sb["biasA1"][:, 0:1],
                                1.0, op0=Alu.add, op1=Alu.min)
                            nc.scalar.activation(
                                V1[0:64, sl], B1[0:64, 0:n], Act.Tanh,
                                bias=bias_sb["biasB1"][0:64, 0:1])
                            nc.gpsimd.tensor_scalar(
                                O2[64:128, 0:n], B1[64:128, 0:n],
                                bias_sb["biasB1"][64:128, 0:1],
                                1.0, op0=Alu.add, op1=Alu.min)
                            nc.vector.scalar_tensor_tensor(
                                st["T1"][:, off:off + n], S1[:, 0:n], 0.0,
                                V1[:, sl], op0=Alu.max, op1=Alu.mult)
                        if p <= T:
                            S0 = spool.tile([128, 512], f32, tag="S0")
                            nc.gpsimd.tensor_scalar(
                                S0[:, 0:n], A0[:, 0:n],
                                bias_sb["biasA0"][:, 0:1],
                                1.0, op0=Alu.add, op1=Alu.min)
                            nc.scalar.activation(
                                V0[64:128, sl], B0[64:128, 0:n], Act.Tanh,
                                bias=bias_sb["biasB0"][64:128, 0:1])
                            nc.gpsimd.tensor_scalar(
                                O2[0:64, 0:n], B0[0:64, 0:n],
                                bias_sb["biasB0"][0:64, 0:1],
                                1.0, op0=Alu.add, op1=Alu.min)
                            nc.vector.scalar_tensor_tensor(
                                st["T0"][:, off:off + n], S0[:, 0:n], 0.0,
                                V0[:, sl], op0=Alu.max, op1=Alu.mult)
                    if p >= 2:
                        nc.sync.dma_start(out=st["X1"][64:128, 0:gn],
                                          in_=st["T1"][0:64, 0:gn])
                    if p <= T:
                        nc.sync.dma_start(out=st["X0"][0:64, 0:gn],
                                          in_=st["T0"][64:128, 0:gn])
                    return st

                def emit_back(pr, st):
                    gch = st["gch"]
                    gn = st["gn"]
                    for ci, (off, cu, n) in enumerate(gch):
                        s = cu - SB
                        sl = slice(s, s + n)
                        if p >= 2:
                            nc.gpsimd.tensor_tensor(
                                V1[64:128, sl], st["T1"][64:128, off:off + n],
                                st["X1"][64:128, off:off + n], op=Alu.add)
                        if p <= T:
                            eng = nc.vector if (pr + ci) % 2 == 0 else nc.gpsimd
                            eng.tensor_tensor(
                                V0[0:64, sl], st["T0"][0:64, off:off + n],
                                st["X0"][0:64, off:off + n], op=Alu.add)
                    s0 = gch[0][1] - SB
                    psl = slice(s0, s0 + gn)
                    if p <= T:
                        nc.scalar.activation(TCt[0:64, psl], V0[0:64, psl],
                                             Act.Tanh)
                    if p >= 2:
                        nc.scalar.activation(TCt[64:128, psl], V1[64:128, psl],
                                             Act.Tanh)
                    for ci, (off, cu, n) in enumerate(gch):
                        s = cu - SB
                        sl = slice(s, s + n)
                        O2 = st["O2s"][ci]
                        if p == 1:
                            nc.vector.scalar_tensor_tensor(
                                Ucur[0:64, cu:cu + n], O2[0:64, 0:n], 0.0,
                                TCt[0:64, sl], op0=Alu.max, op1=Alu.mult)
                        elif p <= T:
                            nc.vector.scalar_tensor_tensor(
                                Ucur[:, cu:cu + n], O2[:, 0:n], 0.0,
                                TCt[:, sl], op0=Alu.max, op1=Alu.mult)
                        if p == T:
                            nc.vector.scalar_tensor_tensor(
                                Hf[:, sl], O2[0:64, 0:n], 0.0, TCt[0:64, sl],
                                op0=Alu.max, op1=Alu.mult)
                        if p == T + 1:
                            nc.gpsimd.scalar_tensor_tensor(
                                Hf[:, sl], O2[64:128, 0:n], 0.0,
                                TCt[64:128, sl], op0=Alu.max, op1=Alu.mult)
                    if p == T:
                        nc.sync.dma_start(out=out_d[0][:, psl], in_=Hf[:, psl])
                        nc.sync.dma_start(out=out_d[1][:, psl],
                                          in_=V0[0:64, psl])
                    if p == T + 1:
                        nc.sync.dma_start(out=out_d[2][:, psl], in_=Hf[:, psl])
                        nc.sync.dma_start(out=out_d[3][:, psl],
                                          in_=V1[64:128, psl])
                    if p <= T and pr in (1, 3, 4):
                        gids = {1: [0, 1, 2, 3], 3: [4, 5, 6, 7], 4: [8]}[pr]
                        glo = CHUNKS[gids[0]][0]
                        ghi = CHUNKS[gids[-1]][0] + CHUNKS[gids[-1]][1]
                        gw = ghi - glo
                        j0 = (glo - 132 + 65) // 66
                        j1 = (ghi - 1 - 132) // 66
                        if j1 >= j0:
                            padap = AP(tensor=Ucur[:, :].tensor,
                                       offset=int(132 + 66 * j0),
                                       ap=[[NB, 128], [66, j1 - j0 + 1], [1, 2]])
                            nc.gpsimd.memset(padap, 0.0)
                        if p <= T - 1:
                            hsrc = Ucur[0:64, glo:glo + gw]
                            nc.sync.dma_start(
                                out=Dn[0:64, glo:glo + gw], in_=hsrc)
                            dst_sh = AP(tensor=Dn[:, :].tensor,
                                        offset=int(64 * NB + glo - 1),
                                        ap=[[NB, 64], [1, gw]])
                            nc.sync.dma_start(out=dst_sh, in_=hsrc)

                # software-pipelined emission: front(pr) || back(pr-1)
                prev = None
                for pr in range(len(PAIRS)):
                    st = emit_front(pr)
                    if prev is not None:
                        emit_back(pr - 1, prev)
                    prev = st
                emit_back(len(PAIRS) - 1, prev)

    nc.compile()
    _EMITTED["nc"] = nc
    return nc


def build_in_maps(x, Wx0, Wh0, b0, Wx1, Wh1, b1):
    packed = pack_weights(np.asarray(Wx0), np.asarray(Wh0), np.asarray(b0),
                          np.asarray(Wx1), np.asarray(Wh1), np.asarray(b1))
    x = np.asarray(x)
    in_maps = []
    for b in range(B):
        m = dict(packed)
        m["x9"] = build_x9(x[b, :, :, :, 0])
        m["zeros"] = np.zeros((128, NB), E4)
        in_maps.append(m)
    return in_maps


def kernel(x, Wx0, Wh0, b0, Wx1, Wh1, b1):
    from concourse.bass_utils import run_bass_kernel_spmd

    nc = _emit()
    in_maps = build_in_maps(x, Wx0, Wh0, b0, Wx1, Wh1, b1)
    res = run_bass_kernel_spmd(nc, in_maps, list(range(B)))

    out = np.zeros((2, 2, B, H, W, F), np.float32)
    for b in range(B):
        r = res.results[b]["out"]  # [4, 64, 4224]
        for (l, si), arr in (((0, 0), r[0]), ((0, 1), r[1]),
                             ((1, 0), r[2]), ((1, 1), r[3])):
            out[l, si, b] = arr.reshape(64, 64, 66)[:, :, 1:65].transpose(1, 2, 0)
    return out
